# revision 20
# baseline (speedup 1.0000x reference)
"""ONGNN (2-layer ordered-neuron GNN) on 8 Trainium2 NeuronCores.

Strategy: shard DESTINATION nodes across the 8 cores (12500/core, padded to
12544 = 98*128).  Edges are bucketed on the host by (core, dst-window-of-128);
per-window message-tile counts are equalized across cores so one SPMD program
serves all cores.  Each conv layer:
  - AllGather of the bf16 node-feature shards -> full table in each core's DRAM
  - indirect-DMA gather of source rows (one 256B row per edge)
  - segment-sum via one-hot matmuls accumulated in PSUM (one-hot built on-chip
    from dst indices with a broadcast is_equal)
  - node-parallel dense math (transition matmul, softmax, cumsum, gating mix,
    layernorm) batched over superwindows of 4x128 nodes.
"""
import sys
import numpy as np

sys.path.insert(0, "/opt/trn_rl_repo")

import concourse.bass as bass
import concourse.bacc as bacc
import concourse.mybir as mybir
import concourse.tile as tile
from concourse import bass_utils

F = 128       # feature dim (IN_C == HID)
CH = 64       # CHUNK
OUT_C = 40
EPS = 1e-5
NCORES = 8

FULL_CFG = dict(N=100000, E=1000000, SH=12500, WPC=98, SW=4)
# SH: dst nodes per core; WPC: 128-node windows per core (ceil(SH/128));
# SW: windows per superwindow (batching factor for elementwise ops).

ABLATE = set()
STREAM_BUFS = 2
TINY_BUFS = 4
AOp = None  # filled lazily
AF = None


def _host_prep(x, edge_index, cfg):
    """Bucket edges by (core, window, src-chunk), build device arrays and the
    shared tile schedule.  Message stream order per superwindow: for each
    source-table chunk, for each window in the superwindow, that (w,c) run's
    tiles (padded to 128).  One dma_gather call covers one (sw, chunk) run."""
    N, E, SH, WPC, SW = cfg["N"], cfg["E"], cfg["SH"], cfg["WPC"], cfg["SW"]
    SHP = WPC * 128
    NFULL = NCORES * SHP
    n_sw = -(-WPC // SW)
    # source-table groups: window-aligned slices of each core's shard; the
    # AllGather is split into one collective per group so it can start as
    # soon as the producing superwindows finish.  8*rows per group must be
    # int16-addressable.  Last group smallest to shorten the serial tail.
    gsw = [(0, 7), (7, 14), (14, 21), (21, n_sw)]
    G_ENDS_W = [min(ge * SW, WPC) for _, ge in gsw]           # [28,56,84,98]
    row_ends = [w * 128 for w in G_ENDS_W]
    row_offs = [0] + row_ends[:-1]
    GROWS = [8 * (e - o) for o, e in zip(row_offs, row_ends)]  # rows per buf
    GOFF = np.concatenate([[0], np.cumsum(GROWS)])[:-1].tolist()
    NCH = len(GROWS)
    assert max(GROWS) <= 32767
    src = np.asarray(edge_index[0], dtype=np.int64)
    dst = np.asarray(edge_index[1], dtype=np.int64)

    core = dst // SH
    dst_loc = dst - core * SH
    win = dst_loc >> 7
    src_core = src // SH
    src_loc = src % SH
    chunk = np.searchsorted(np.asarray(row_ends), src_loc, side="right")
    # row inside the group's 8-core gather buffer
    grow = (src_core * np.asarray([e - o for o, e in zip(row_offs, row_ends)])[chunk]
            + (src_loc - np.asarray(row_offs)[chunk]))
    bucket = ((core * WPC + win) * NCH + chunk).astype(np.int64)
    order = np.argsort(bucket, kind="stable")
    bcnt = np.bincount(bucket, minlength=NCORES * WPC * NCH) \
        .reshape(NCORES, WPC, NCH)
    tpwc = -(-bcnt // 128)
    tpwc = tpwc.max(axis=0)                      # [WPC, NCH]
    for w in range(WPC):
        if tpwc[w].sum() == 0:
            tpwc[w, 0] = 1

    # schedule: per sw, per chunk, the window runs; global tile offsets
    sched = []
    t_acc = 0
    tile_win = []                                # window id of every tile
    for sw in range(n_sw):
        w0, w1 = sw * SW, min((sw + 1) * SW, WPC)
        runs = []
        for c in range(NCH):
            wruns = [(w, int(tpwc[w, c])) for w in range(w0, w1)
                     if tpwc[w, c] > 0]
            nt = sum(n for _, n in wruns)
            if nt == 0:
                continue
            runs.append((c, t_acc, wruns))
            for w, n in wruns:
                tile_win.extend([w] * n)
            t_acc += nt
        sched.append(runs)
    T = t_acc

    starts = np.zeros(NCORES * WPC * NCH + 1, np.int64)
    np.cumsum(bcnt.reshape(-1), out=starts[1:])
    idx16 = np.zeros((NCORES, T * 128), np.int16)
    rel = np.full((NCORES, T * 128), 512.0, np.float32)
    for k in range(NCORES):
        for sw in range(n_sw):
            for c, t0, wruns in sched[sw]:
                pos = t0 * 128
                for w, ntile in wruns:
                    b = (k * WPC + w) * NCH + c
                    sel = order[starts[b]:starts[b + 1]]
                    sel = sel[np.argsort(grow[sel], kind="stable")]
                    n = sel.size
                    idx16[k, pos:pos + n] = grow[sel].astype(np.int16)
                    rel[k, pos:pos + n] = (dst_loc[sel] - w * 128) \
                        .astype(np.float32)
                    pos += ntile * 128
    # device layouts
    idx16_dev = np.ascontiguousarray(idx16.reshape(NCORES, T * 8, 16)
                                     .transpose(0, 2, 1))     # [NC,16,T*8]
    idx16_dev = np.tile(idx16_dev, (1, 8, 1))                 # [NC,128,T*8]
    rel_dev = np.ascontiguousarray(rel.reshape(NCORES, T, 128)
                                   .transpose(0, 2, 1))       # [NC,128,T]
    # each rel value stored twice so the on-chip is_equal can use a
    # contiguous pair view (DVE 2x mode needs last-dim stride 1)
    rel_dev = np.repeat(rel_dev, 2, axis=2)                   # [NC,128,2T]

    cnt = np.bincount(dst, minlength=N).astype(np.float32)
    inv = 1.0 / np.maximum(cnt, 1.0)
    inv_dev = np.ones((NCORES, SHP), np.float32)
    for k in range(NCORES):
        inv_dev[k, :SH] = inv[k * SH:(k + 1) * SH]
    inv_dev = inv_dev.reshape(NCORES, WPC, 128).transpose(0, 2, 1)
    inv_dev = np.ascontiguousarray(inv_dev)

    xT_dev = np.zeros((NCORES, F, SHP), np.float32)
    x = np.asarray(x, np.float32)
    for k in range(NCORES):
        xT_dev[k, :, :SH] = x[k * SH:(k + 1) * SH].T

    return dict(idx16=idx16_dev, rel=rel_dev, inv=inv_dev, xT=xT_dev,
                sched=sched, tile_win=tile_win, T=T, SHP=SHP,
                GROWS=GROWS, GOFF=GOFF, row_offs=row_offs, row_ends=row_ends,
                gsw=gsw, tpwc=tpwc)


def _affine_trivial(g, b):
    return bool(np.allclose(g, 1.0, atol=1e-7) and np.allclose(b, 0.0, atol=1e-7))


def build_nc(cfg, prep, flags, debug_dumps=False, no_collectives=False, repeat=1):
    """Build the SPMD Bass program. flags: dict of *_trivial booleans."""
    global AOp, AF
    AOp = mybir.AluOpType
    AF = mybir.ActivationFunctionType
    dt = mybir.dt

    WPC, SW = cfg["WPC"], cfg["SW"]
    SHP = WPC * 128
    T = prep["T"]
    sched, tile_win = prep["sched"], prep["tile_win"]
    GROWS, GOFF = prep["GROWS"], prep["GOFF"]
    row_offs, row_ends, gsw = prep["row_offs"], prep["row_ends"], prep["gsw"]
    NFULL = NCORES * SHP

    nc = bacc.Bacc("TRN2", target_bir_lowering=False, debug=False,
                   num_devices=NCORES, num_swdge_queues=4)

    # ---- I/O ----
    xT_d = nc.dram_tensor("xT", [F, SHP], dt.bfloat16, kind="ExternalInput")
    win_d = nc.dram_tensor("w_in", [F, F], dt.bfloat16, kind="ExternalInput")
    idx_d = nc.dram_tensor("idx16", [128, T * 8], dt.int16, kind="ExternalInput")
    rel_d = nc.dram_tensor("rel", [128, 2 * T], dt.bfloat16, kind="ExternalInput")
    inv_d = nc.dram_tensor("inv", [128, WPC], dt.float32, kind="ExternalInput")
    tmw0_d = nc.dram_tensor("tmw0", [2 * F, CH], dt.bfloat16, kind="ExternalInput")
    tmw1_d = nc.dram_tensor("tmw1", [2 * F, CH], dt.bfloat16, kind="ExternalInput")
    wout_d = nc.dram_tensor("wout", [F, OUT_C], dt.bfloat16, kind="ExternalInput")
    iota_d = nc.dram_tensor("iota", [128, 128], dt.bfloat16, kind="ExternalInput")
    ident_d = nc.dram_tensor("ident", [128, 128], dt.bfloat16, kind="ExternalInput")
    out_d = nc.dram_tensor("out", [SHP, OUT_C], dt.float32, kind="ExternalOutput")
    gb_d = {}
    for nm in ("bin", "gin", "bein", "lng0", "lnb0", "lng1", "lnb1", "tmb0",
               "tmb1", "bout"):
        if not flags[nm + "_triv"]:
            width = {"tmb0": CH, "tmb1": CH, "bout": OUT_C}.get(nm, F)
            gb_d[nm] = nc.dram_tensor(nm, [128, width], dt.float32,
                                      kind="ExternalInput")

    dbg = {}
    if debug_dumps:
        for nm, shape, d in (("dbg_h0", [SHP, F], dt.float32),
                             ("dbg_hf0", [NFULL, F], dt.bfloat16),
                             ("dbg_m0", [SHP, F], dt.float32),
                             ("dbg_sig0", [128, WPC * CH], dt.float32),
                             ("dbg_g0", [128, 0], dt.bfloat16),
                             ("dbg_oh0", [128, 0], dt.bfloat16),
                             ("dbg_h1", [SHP, F], dt.float32)):
            if nm in ("dbg_g0", "dbg_oh0"):
                shape = [128, T * 128]
            dbg[nm] = nc.dram_tensor(nm, shape, d, kind="ExternalOutput")
    bounce0 = nc.dram_tensor("bounce0", [SHP, F], dt.bfloat16)
    bounce1 = nc.dram_tensor("bounce1", [SHP, F], dt.bfloat16)
    hfull0 = nc.dram_tensor("hfull0", [NFULL, F], dt.bfloat16, addr_space="Shared")
    hfull1 = nc.dram_tensor("hfull1", [NFULL, F], dt.bfloat16, addr_space="Shared")

    n_sw = (WPC + SW - 1) // SW

    with tile.TileContext(nc) as tc:
        import contextlib
        ctx = contextlib.ExitStack()
        with ctx:
            ctx.enter_context(nc.allow_low_precision(
                reason="bf16 elementwise; LN stats accumulate in fp32"))
            res = ctx.enter_context(tc.tile_pool(name="res", bufs=1))
            stream = ctx.enter_context(tc.tile_pool(name="stream", bufs=STREAM_BUFS))
            tiny = ctx.enter_context(tc.tile_pool(name="tiny", bufs=TINY_BUFS))
            small = ctx.enter_context(tc.tile_pool(name="small", bufs=3))
            psum = ctx.enter_context(tc.tile_pool(name="psum", bufs=2, space="PSUM"))
            pst = ctx.enter_context(tc.tile_pool(name="pst", bufs=2, space="PSUM"))

            # ---- residents / constants ----
            hT = res.tile([128, SHP], dt.bfloat16, tag="hT")       # feat-major own shard
            h_own = res.tile([128, SHP], dt.bfloat16, tag="h_own") # node-major own shard
            sigc = res.tile([128, WPC * CH], dt.bfloat16, tag="sigc")
            rel_t = res.tile([128, 2 * T], dt.bfloat16, tag="rel")
            inv_t = res.tile([128, WPC], dt.float32, tag="inv")
            w_in = res.tile([F, F], dt.bfloat16, tag="w_in")
            tmw = [[res.tile([F, CH], dt.bfloat16, tag=f"tmw{l}{h}", name=f"tmw{l}{h}")
                    for h in range(2)] for l in range(2)]
            wout = res.tile([F, OUT_C], dt.bfloat16, tag="wout")
            iota_t = res.tile([128, 128], dt.bfloat16, tag="iota")
            ident = res.tile([128, 128], dt.bfloat16, tag="ident")
            gb_t = {}
            for nm, d in gb_d.items():
                gb_t[nm] = res.tile(list(d.shape), dt.float32, tag=nm, name=nm)
                nc.sync.dma_start(gb_t[nm][:], d.ap())

            nc.sync.dma_start(rel_t[:], rel_d.ap())
            nc.sync.dma_start(inv_t[:], inv_d.ap())
            nc.sync.dma_start(w_in[:], win_d.ap())
            for l, d in enumerate((tmw0_d, tmw1_d)):
                nc.sync.dma_start(tmw[l][0][:], d.ap()[0:F, :])
                nc.sync.dma_start(tmw[l][1][:], d.ap()[F:2 * F, :])
            nc.sync.dma_start(wout[:], wout_d.ap())
            nc.sync.dma_start(iota_t[:], iota_d.ap())
            nc.sync.dma_start(ident[:], ident_d.ap())
            eps_t = res.tile([128, 1], dt.float32, tag="eps")
            nc.vector.memset(eps_t[:], EPS)

            def ln_smalls(su, sq, nw):
                """su,sq: [128,nw] fp32 -> (mean, rs) each [128,nw] fp32"""
                mean = small.tile([128, nw], dt.float32, tag="mean")
                nc.vector.tensor_scalar(mean[:], su, 1.0 / F, None, AOp.mult)
                t1 = small.tile([128, nw], dt.float32, tag="t1")
                nc.vector.tensor_tensor(t1[:], mean[:], su, AOp.mult)
                t2 = small.tile([128, nw], dt.float32, tag="t2")
                nc.vector.tensor_tensor(t2[:], sq, t1[:], AOp.subtract)
                srt = small.tile([128, nw], dt.float32, tag="srt")
                nc.scalar.activation(srt[:], t2[:], AF.Sqrt, bias=eps_t[:],
                                     scale=1.0 / F)
                rs = small.tile([128, nw], dt.float32, tag="rs")
                nc.vector.reciprocal(rs[:], srt[:])
                return mean, rs

            def apply_ln(dst_ap, u_ap, mean, rs, nw, gnm, bnm):
                """dst = (u - mean)*rs [* g] [+ b]; u_ap/dst [128, nw*128] bf16.
                Per-window fused tensor_scalar keeps operands contiguous
                (2x DVE mode); scalars are per-partition APs."""
                if gnm is None and bnm is None:
                    for j in range(nw):
                        nc.vector.tensor_scalar(
                            dst_ap[:, j * 128:(j + 1) * 128],
                            u_ap[:, j * 128:(j + 1) * 128],
                            mean[:, j:j + 1], rs[:, j:j + 1],
                            AOp.subtract, AOp.mult)
                    return
                u3 = u_ap.rearrange("p (w f) -> p w f", w=nw)
                d3 = dst_ap.rearrange("p (w f) -> p w f", w=nw)
                mb = mean[:].unsqueeze(2).broadcast_to([128, nw, 128])
                rb = rs[:].unsqueeze(2).broadcast_to([128, nw, 128])
                nc.vector.tensor_tensor(d3, u3, mb, AOp.subtract)
                nc.vector.tensor_tensor(d3, d3, rb, AOp.mult)
                if gnm is not None:
                    g3 = gb_t[gnm][:].unsqueeze(1).broadcast_to([128, nw, 128])
                    nc.vector.tensor_tensor(d3, d3, g3, AOp.mult)
                if bnm is not None:
                    b3 = gb_t[bnm][:].unsqueeze(1).broadcast_to([128, nw, 128])
                    nc.vector.tensor_tensor(d3, d3, b3, AOp.add)

            def _ag_group(bounce, hfull, g):
                r0, r1 = row_offs[g], row_ends[g]
                if no_collectives:
                    nc.sync.dma_start(hfull.ap()[GOFF[g]:GOFF[g] + (r1 - r0), :],
                                      bounce.ap()[r0:r1, :])
                else:
                    nc.gpsimd.collective_compute(
                        "AllGather", AOp.bypass,
                        replica_groups=[list(range(NCORES))],
                        ins=[bounce.ap()[r0:r1, :].opt()],
                        outs=[hfull.ap()[GOFF[g]:GOFF[g] + GROWS[g], :].opt()])

            def _once():
                # ============ Phase A: input MLP (h0 = LN(relu(x W + b))) ============
                for sw in range(n_sw):
                    w0 = sw * SW
                    nw = min(SW, WPC - w0)
                    nwf = nw * 128
                    ps_z = psum.tile([128, SW * 128], dt.float32, tag="ps_acc")
                    xt4 = stream.tile([128, SW * 128], dt.bfloat16, tag="m4", name="xt4")
                    nc.sync.dma_start(xt4[:, :nwf],
                                      xT_d.ap()[:, w0 * 128:w0 * 128 + nwf])
                    for j in range(nw):
                        nc.tensor.matmul(ps_z[:, j * 128:(j + 1) * 128],
                                         xt4[:, j * 128:(j + 1) * 128],
                                         w_in[:], start=True, stop=True)
                    # relu (+ optional b_in) -> r
                    r4 = stream.tile([128, SW * 128], dt.bfloat16, tag="u4", name="r4")
                    if "bin" in gb_t:
                        b3 = gb_t["bin"][:].unsqueeze(1).broadcast_to([128, nw, 128])
                        z3 = ps_z[:, :nwf].rearrange("p (w f) -> p w f", w=nw)
                        nc.vector.tensor_tensor(z3, z3, b3, AOp.add)
                    nc.scalar.activation(r4[:, :nwf], ps_z[:, :nwf], AF.Relu)
                    su = small.tile([128, SW], dt.float32, tag="su")
                    nc.vector.tensor_reduce(
                        su[:, :nw], r4[:, :nwf].rearrange("p (w f) -> p w f", w=nw),
                        mybir.AxisListType.X, AOp.add)
                    sqs = stream.tile([128, SW * 128], dt.bfloat16, tag="e4", name="sqs")
                    nc.scalar.activation(sqs[:, :nwf], r4[:, :nwf], AF.Square)
                    sq = small.tile([128, SW], dt.float32, tag="sq")
                    nc.vector.tensor_reduce(
                        sq[:, :nw], sqs[:, :nwf].rearrange("p (w f) -> p w f", w=nw),
                        mybir.AxisListType.X, AOp.add)
                    mean, rs = ln_smalls(su[:, :nw], sq[:, :nw], nw)
                    apply_ln(h_own[:, w0 * 128:w0 * 128 + nwf], r4[:, :nwf],
                             mean, rs, nw,
                             "gin" if "gin" in gb_t else None,
                             "bein" if "bein" in gb_t else None)
                    nc.sync.dma_start(
                        bounce0.ap()[w0 * 128:w0 * 128 + nwf, :]
                            .rearrange("(w p) f -> p w f", w=nw),
                        h_own[:, w0 * 128:w0 * 128 + nwf]
                            .rearrange("p (w f) -> p w f", w=nw))
                    ps_t4 = pst.tile([128, SW * 128], dt.bfloat16, tag="ps_t")
                    for j in range(nw):
                        w = w0 + j
                        nc.tensor.transpose(ps_t4[:, j * 128:(j + 1) * 128],
                                            h_own[:, w * 128:(w + 1) * 128], ident[:])
                    nc.scalar.activation(hT[:, w0 * 128:w0 * 128 + nwf],
                                         ps_t4[:, :nwf], AF.Copy)
                    for g, (gs, ge) in enumerate(gsw):
                        if sw == ge - 1:
                            _ag_group(bounce0, hfull0, g)

                if debug_dumps:
                    nc.sync.dma_start(
                        dbg["dbg_h0"].ap().rearrange("(w p) f -> p w f", w=WPC),
                        h_own[:].rearrange("p (w f) -> p w f", w=WPC))
                if debug_dumps:
                    nc.sync.dma_start(dbg["dbg_hf0"].ap(), hfull0.ap())

                # ============ Phase B/C: conv layers ============
                for l in range(2):
                    hfull = (hfull0, hfull1)[l]
                    for sw in range(n_sw):
                        w0 = sw * SW
                        nw = min(SW, WPC - w0)
                        nwf = nw * 128
                        runs = sched[sw]
                        c0 = runs[0][1]
                        c1 = runs[-1][1] + sum(n for _, n in runs[-1][2])
                        TS = c1 - c0
                        # gather (one dma_gather per source chunk) + onehot
                        g_t = stream.tile([128, TS * 128], dt.bfloat16, tag="g")
                        idx_sw = tiny.tile([128, TS * 8], dt.int16, tag="idx_sw")
                        nc.sync.dma_start(idx_sw[:], idx_d.ap()[:, c0 * 8:c1 * 8])
                        if "gather" in ABLATE:
                            nc.sync.dma_start(
                                g_t[:].rearrange("p (t f) -> p t f", t=TS),
                                hfull.ap()[0:TS * 128, :]
                                    .rearrange("(t p) f -> p t f", p=128))
                        for gi, (c, t0, wruns) in enumerate(
                                [] if "gather" in ABLATE else runs):
                            nt = sum(n for _, n in wruns)
                            nc.gpsimd.dma_gather(
                                g_t[:, (t0 - c0) * 128:(t0 - c0 + nt) * 128]
                                    .rearrange("p (t f) -> p t f", t=nt),
                                hfull.ap()[GOFF[c]:GOFF[c] + GROWS[c], :],
                                idx_sw[:, (t0 - c0) * 8:(t0 - c0 + nt) * 8],
                                nt * 128, nt * 128, F, single_packet=False,
                                queue_num=gi % 4)
                        oh_t = stream.tile([128, TS * 128], dt.bfloat16, tag="oh")
                        # pair views keep every last dim stride-1 so the DVE
                        # runs is_equal in 2x mode
                        oh4 = oh_t[:].rearrange("p (t f2 r) -> p t f2 r",
                                                t=TS, r=2)
                        iota_b = iota_t[:].rearrange("p (f2 r) -> p f2 r", r=2) \
                            .unsqueeze(1).broadcast_to([128, TS, 64, 2])
                        rel_b = rel_t[:, 2 * c0:2 * c1] \
                            .rearrange("p (t r) -> p t r", r=2) \
                            .unsqueeze(2).broadcast_to([128, TS, 64, 2])
                        if "onehot" not in ABLATE:
                            nc.vector.tensor_tensor(oh4, iota_b, rel_b, AOp.is_equal)
                        else:
                            nc.vector.memset(oh_t[:], 0.0)
                        if debug_dumps and l == 0:
                            nc.sync.dma_start(dbg["dbg_g0"].ap()[:, c0 * 128:c1 * 128],
                                              g_t[:])
                            nc.sync.dma_start(dbg["dbg_oh0"].ap()[:, c0 * 128:c1 * 128],
                                              oh_t[:])
                        # segment matmuls, window-major so each window's PSUM
                        # accumulation group opens and closes contiguously
                        ps_m = psum.tile([128, SW * 128], dt.float32, tag="ps_acc")
                        for j in range(nw):
                            w = w0 + j
                            wtiles = [t for t in range(c0, c1) if tile_win[t] == w]
                            if "seg" in ABLATE:
                                wtiles = wtiles[:1]
                            for i, t in enumerate(wtiles):
                                tl = t - c0
                                nc.tensor.matmul(
                                    ps_m[:, j * 128:(j + 1) * 128],
                                    oh_t[:, tl * 128:(tl + 1) * 128],
                                    g_t[:, tl * 128:(tl + 1) * 128],
                                    start=(i == 0), stop=(i == len(wtiles) - 1))
                        # m (scaled) node-major
                        m4 = stream.tile([128, SW * 128], dt.bfloat16, tag="m4")
                        m3 = m4[:, :nwf].rearrange("p (w f) -> p w f", w=nw)
                        iv = inv_t[:, w0:w0 + nw].unsqueeze(2).broadcast_to(
                            [128, nw, 128])
                        nc.vector.tensor_tensor(
                            m3, ps_m[:, :nwf].rearrange("p (w f) -> p w f", w=nw),
                            iv, AOp.mult)
                        if debug_dumps and l == 0:
                            nc.sync.dma_start(
                                dbg["dbg_m0"].ap()[w0 * 128:w0 * 128 + nwf, :]
                                    .rearrange("(w p) f -> p w f", w=nw),
                                m3)
                        # mT batched transpose + tm matmuls
                        ps_t4m = pst.tile([128, SW * 128], dt.bfloat16, tag="ps_t")
                        for j in range(nw):
                            nc.tensor.transpose(ps_t4m[:, j * 128:(j + 1) * 128],
                                                m4[:, j * 128:(j + 1) * 128],
                                                ident[:])
                        mt4 = tiny.tile([128, SW * 128], dt.bfloat16, tag="mt")
                        nc.scalar.activation(mt4[:, :nwf], ps_t4m[:, :nwf], AF.Copy)
                        ps_tm = psum.tile([128, SW * CH], dt.float32, tag="ps_sm")
                        for j in range(nw):
                            w = w0 + j
                            nc.tensor.matmul(ps_tm[:, j * CH:(j + 1) * CH],
                                             hT[:, w * 128:(w + 1) * 128],
                                             tmw[l][0][:], start=True, stop=False)
                            nc.tensor.matmul(ps_tm[:, j * CH:(j + 1) * CH],
                                             mt4[:, j * 128:(j + 1) * 128],
                                             tmw[l][1][:],
                                             start=False, stop=True)
                        nwc = nw * CH
                        if ("tmb0", "tmb1")[l] in gb_t:
                            tb = gb_t[("tmb0", "tmb1")[l]][:].unsqueeze(1) \
                                .broadcast_to([128, nw, CH])
                            z3 = ps_tm[:, :nwc].rearrange("p (w c) -> p w c", w=nw)
                            nc.vector.tensor_tensor(z3, z3, tb, AOp.add)
                        # softmax (no max-sub) + cumsum
                        e4 = stream.tile([128, SW * CH], dt.float32, tag="e4")
                        nc.scalar.activation(e4[:, :nwc], ps_tm[:, :nwc], AF.Exp)
                        s4 = small.tile([128, SW], dt.float32, tag="s4")
                        nc.vector.tensor_reduce(
                            s4[:, :nw], e4[:, :nwc].rearrange("p (w c) -> p w c", w=nw),
                            mybir.AxisListType.X, AOp.add)
                        r4s = small.tile([128, SW], dt.float32, tag="r4s")
                        nc.vector.reciprocal(r4s[:, :nw], s4[:, :nw])
                        cs4 = stream.tile([128, SW * CH], dt.float32, tag="cs4")
                        for j in range(nw):
                            nc.vector.tensor_tensor_scan(
                                cs4[:, j * CH:(j + 1) * CH],
                                e4[:, j * CH:(j + 1) * CH],
                                e4[:, j * CH:(j + 1) * CH], 0.0, AOp.add, AOp.bypass)
                        # sig update
                        rb = r4s[:, :nw].unsqueeze(2).broadcast_to([128, nw, CH])
                        cs3 = cs4[:, :nwc].rearrange("p (w c) -> p w c", w=nw)
                        sg_cols = sigc[:, w0 * CH:w0 * CH + nwc]
                        sg3 = sg_cols.rearrange("p (w c) -> p w c", w=nw)
                        if l == 0:
                            nc.vector.tensor_tensor(sg3, cs3, rb, AOp.mult)
                            sig_src = sg_cols
                        else:
                            t4 = stream.tile([128, SW * CH], dt.bfloat16, tag="t4")
                            t3 = t4[:, :nwc].rearrange("p (w c) -> p w c", w=nw)
                            nc.vector.tensor_tensor(t3, cs3, rb, AOp.mult)
                            a4 = stream.tile([128, SW * CH], dt.bfloat16, tag="a4")
                            nc.vector.tensor_tensor(a4[:, :nwc], sg_cols, t4[:, :nwc],
                                                    AOp.mult)
                            nc.vector.tensor_tensor(t4[:, :nwc], t4[:, :nwc],
                                                    a4[:, :nwc], AOp.subtract)
                            nc.vector.tensor_tensor(t4[:, :nwc], t4[:, :nwc],
                                                    sg_cols, AOp.add)
                            sig_src = t4[:, :nwc]
                        # mix u = h*sig + m*(1-sig) = m + sig*(h-m)
                        # sig broadcast over the repeat axis (64 -> 128)
                        sig_b = sig_src.rearrange("p (w c) -> p w c", w=nw) \
                            .unsqueeze(3).broadcast_to([128, nw, CH, 2])
                        hcols = h_own[:, w0 * 128:w0 * 128 + nwf]
                        u4 = stream.tile([128, SW * 128], dt.bfloat16, tag="u4")
                        nc.vector.tensor_tensor(u4[:, :nwf], hcols, m4[:, :nwf],
                                                AOp.subtract)
                        u4d = u4[:, :nwf].rearrange("p (w c r) -> p w c r",
                                                    w=nw, r=2)
                        nc.vector.tensor_tensor(u4d, u4d, sig_b, AOp.mult)
                        nc.vector.tensor_tensor(u4[:, :nwf], u4[:, :nwf], m4[:, :nwf],
                                                AOp.add)
                        # LN stats
                        su = small.tile([128, SW], dt.float32, tag="su")
                        nc.vector.tensor_reduce(
                            su[:, :nw], u4[:, :nwf].rearrange("p (w f) -> p w f", w=nw),
                            mybir.AxisListType.X, AOp.add)
                        sqs = stream.tile([128, SW * 128], dt.bfloat16, tag="e4", name="sqs")
                        nc.scalar.activation(sqs[:, :nwf], u4[:, :nwf], AF.Square)
                        sq = small.tile([128, SW], dt.float32, tag="sq")
                        nc.vector.tensor_reduce(
                            sq[:, :nw], sqs[:, :nwf].rearrange("p (w f) -> p w f", w=nw),
                            mybir.AxisListType.X, AOp.add)
                        mean, rs = ln_smalls(su[:, :nw], sq[:, :nw], nw)
                        gnm = ("lng0", "lng1")[l]
                        bnm = ("lnb0", "lnb1")[l]
                        if l == 0:
                            apply_ln(hcols, u4[:, :nwf], mean, rs, nw,
                                     gnm if gnm in gb_t else None,
                                     bnm if bnm in gb_t else None)
                            nc.sync.dma_start(
                                bounce1.ap()[w0 * 128:w0 * 128 + nwf, :]
                                    .rearrange("(w p) f -> p w f", w=nw),
                                hcols.rearrange("p (w f) -> p w f", w=nw))
                            ps_t4 = pst.tile([128, SW * 128], dt.bfloat16, tag="ps_t")
                            for j in range(nw):
                                w = w0 + j
                                nc.tensor.transpose(
                                    ps_t4[:, j * 128:(j + 1) * 128],
                                    h_own[:, w * 128:(w + 1) * 128], ident[:])
                            nc.scalar.activation(hT[:, w0 * 128:w0 * 128 + nwf],
                                                 ps_t4[:, :nwf], AF.Copy)
                            for g, (gs, ge) in enumerate(gsw):
                                if sw == ge - 1:
                                    _ag_group(bounce1, hfull1, g)
                        else:
                            h2 = stream.tile([128, SW * 128], dt.bfloat16, tag="hx", name="h2")
                            apply_ln(h2[:, :nwf], u4[:, :nwf], mean, rs, nw,
                                     gnm if gnm in gb_t else None,
                                     bnm if bnm in gb_t else None)
                            ob = stream.tile([128, SW * OUT_C], dt.float32, tag="ob")
                            ps_o = psum.tile([128, SW * OUT_C], dt.float32, tag="ps_sm")
                            ps_t4 = pst.tile([128, SW * 128], dt.bfloat16, tag="ps_t")
                            for j in range(nw):
                                nc.tensor.transpose(
                                    ps_t4[:, j * 128:(j + 1) * 128],
                                    h2[:, j * 128:(j + 1) * 128], ident[:])
                            h2t4 = tiny.tile([128, SW * 128], dt.bfloat16, tag="h2t")
                            nc.scalar.activation(h2t4[:, :nwf], ps_t4[:, :nwf],
                                                 AF.Copy)
                            for j in range(nw):
                                nc.tensor.matmul(ps_o[:, j * OUT_C:(j + 1) * OUT_C],
                                                 h2t4[:, j * 128:(j + 1) * 128],
                                                 wout[:], start=True, stop=True)
                            nwo = nw * OUT_C
                            if "bout" in gb_t:
                                bb = gb_t["bout"][:].unsqueeze(1).broadcast_to(
                                    [128, nw, OUT_C])
                                o3 = ob[:, :nwo].rearrange("p (w o) -> p w o", w=nw)
                                nc.vector.tensor_tensor(
                                    o3, ps_o[:, :nwo].rearrange("p (w o) -> p w o", w=nw),
                                    bb, AOp.add)
                            else:
                                nc.vector.tensor_copy(ob[:, :nwo], ps_o[:, :nwo])
                            nc.sync.dma_start(
                                out_d.ap()[w0 * 128:w0 * 128 + nwf, :]
                                    .rearrange("(w p) o -> p w o", w=nw),
                                ob[:, :nwo].rearrange("p (w o) -> p w o", w=nw))
                    if l == 0 and debug_dumps:
                        nc.sync.dma_start(dbg["dbg_sig0"].ap(), sigc[:])
                        nc.sync.dma_start(
                            dbg["dbg_h1"].ap().rearrange("(w p) f -> p w f", w=WPC),
                            h_own[:].rearrange("p (w f) -> p w f", w=WPC))


            for _rep in range(repeat):
                _once()

    nc.compile()
    return nc


_CACHE = {}


def _sched_key(prep):
    return (tuple(prep["tile_win"]),
            tuple((c, t0, tuple(wr)) for sw in prep["sched"]
                  for c, t0, wr in sw))


def _get_compiled(cfg, prep, flags):
    key = (_sched_key(prep), tuple(sorted(flags.items())))
    if key not in _CACHE:
        _CACHE[key] = build_nc(cfg, prep, flags)
    return _CACHE[key]


class PjrtRunner:
    """Persistent jitted shard_map executor for one compiled nc (8 cores)."""

    def __init__(self, nc, donate=True):
        import jax
        from jax.experimental.shard_map import shard_map
        from jax.sharding import Mesh, PartitionSpec
        from concourse import bass2jax

        bass2jax.install_neuronx_cc_hook()
        self.nc = nc
        in_names, out_names, out_avals, zero_outs = [], [], [], []
        partition_name = (nc.partition_id_tensor.name
                          if nc.partition_id_tensor else None)
        for alloc in nc.m.functions[0].allocations:
            if not isinstance(alloc, mybir.MemoryLocationSet):
                continue
            name = alloc.memorylocations[0].name
            if alloc.kind == "ExternalInput":
                if name != partition_name:
                    in_names.append(name)
            elif alloc.kind == "ExternalOutput":
                import jax.core as jcore
                out_names.append(name)
                aval = jax.core.ShapedArray(
                    tuple(alloc.tensor_shape), mybir.dt.np(alloc.dtype))
                out_avals.append(aval)
                zero_outs.append(np.zeros(alloc.tensor_shape,
                                          mybir.dt.np(alloc.dtype)))
        self.n_params = len(in_names)
        self.out_names = list(out_names)
        self.zero_outs = zero_outs
        all_in = in_names + out_names
        if partition_name is not None:
            all_in.append(partition_name)
        self.in_names_data = in_names

        def _body(*args):
            operands = list(args)
            if partition_name is not None:
                operands.append(bass2jax.partition_id_tensor())
            outs = bass2jax._bass_exec_p.bind(
                *operands,
                out_avals=tuple(out_avals),
                in_names=tuple(all_in),
                out_names=tuple(out_names),
                lowering_input_output_aliases=(),
                sim_require_finite=True,
                sim_require_nnan=True,
                nc=nc,
            )
            return tuple(outs)

        devices = jax.devices()[:NCORES]
        self.mesh = Mesh(np.asarray(devices), ("core",))
        n_out = len(out_names)
        donate_nums = (tuple(range(self.n_params, self.n_params + n_out))
                       if donate else ())
        in_specs = (PartitionSpec("core"),) * (self.n_params + n_out)
        out_specs = (PartitionSpec("core"),) * n_out
        self.fn = jax.jit(
            shard_map(_body, mesh=self.mesh, in_specs=in_specs,
                      out_specs=out_specs, check_rep=False),
            donate_argnums=donate_nums, keep_unused=True)

    def concat_inputs(self, in_maps):
        return [
            np.concatenate([np.asarray(in_maps[c][nm]) for c in range(NCORES)],
                           axis=0)
            for nm in self.in_names_data
        ]

    def zeros(self):
        return [np.zeros((NCORES * z.shape[0], *z.shape[1:]), z.dtype)
                for z in self.zero_outs]

    def __call__(self, concat_in, zeros):
        import jax
        outs = self.fn(*concat_in, *zeros)
        return {nm: np.asarray(outs[i]) for i, nm in enumerate(self.out_names)}


_RUNNERS = {}


def get_runner(cfg, prep, flags):
    key = (_sched_key(prep), tuple(sorted(flags.items())))
    if key not in _RUNNERS:
        _RUNNERS[key] = PjrtRunner(_get_compiled(cfg, prep, flags))
    return _RUNNERS[key]


def run(inputs, cfg):
    x = np.asarray(inputs["x"], np.float32)
    prep = _host_prep(x, np.asarray(inputs["edge_index"]), cfg)
    SH = cfg["SH"]

    flags = make_flags(inputs)
    runner = get_runner(cfg, prep, flags)
    in_maps = make_in_maps(inputs, prep, flags)
    out = runner(runner.concat_inputs(in_maps), runner.zeros())["out"]
    SHP = prep["SHP"]
    out = out.reshape(NCORES, SHP, OUT_C)[:, :SH, :]
    return np.ascontiguousarray(out.reshape(NCORES * SH, OUT_C), dtype=np.float32)


def make_flags(inputs):
    return {
        "bin_triv": _affine_trivial(1.0, inputs["b_in"]),
        "gin_triv": _affine_trivial(inputs["g_in"], 0.0),
        "bein_triv": _affine_trivial(1.0, inputs["be_in"]),
        "lng0_triv": _affine_trivial(inputs["ln_g0"], 0.0),
        "lnb0_triv": _affine_trivial(1.0, inputs["ln_b0"]),
        "lng1_triv": _affine_trivial(inputs["ln_g1"], 0.0),
        "lnb1_triv": _affine_trivial(1.0, inputs["ln_b1"]),
        "tmb0_triv": _affine_trivial(1.0, inputs["tm_b0"]),
        "tmb1_triv": _affine_trivial(1.0, inputs["tm_b1"]),
        "bout_triv": _affine_trivial(1.0, inputs["b_out"]),
    }


def make_in_maps(inputs, prep, flags):
    def bc(v, width):
        return np.tile(np.asarray(v, np.float32).reshape(1, width), (128, 1))

    in_maps = []
    for k in range(NCORES):
        import ml_dtypes
        bf16 = ml_dtypes.bfloat16
        m = {
            "xT": prep["xT"][k].astype(bf16),
            "w_in": np.asarray(inputs["W_in"], np.float32).astype(bf16),
            "idx16": prep["idx16"][k],
            "rel": prep["rel"][k].astype(bf16),
            "inv": prep["inv"][k],
            "tmw0": np.asarray(inputs["tm_W0"], np.float32).astype(bf16),
            "tmw1": np.asarray(inputs["tm_W1"], np.float32).astype(bf16),
            "wout": np.asarray(inputs["W_out"], np.float32).astype(bf16),
            "iota": np.tile(np.arange(128, dtype=np.float32)[None, :],
                            (128, 1)).astype(bf16),
            "ident": np.eye(128, dtype=np.float32).astype(bf16),
        }
        if not flags["bin_triv"]:
            m["bin"] = bc(inputs["b_in"], F)
        if not flags["gin_triv"]:
            m["gin"] = bc(inputs["g_in"], F)
        if not flags["bein_triv"]:
            m["bein"] = bc(inputs["be_in"], F)
        for nm, src in (("lng0", "ln_g0"), ("lnb0", "ln_b0"),
                        ("lng1", "ln_g1"), ("lnb1", "ln_b1")):
            if not flags[nm + "_triv"]:
                m[nm] = bc(inputs[src], F)
        if not flags["tmb0_triv"]:
            m["tmb0"] = bc(inputs["tm_b0"], CH)
        if not flags["tmb1_triv"]:
            m["tmb1"] = bc(inputs["tm_b1"], CH)
        if not flags["bout_triv"]:
            m["bout"] = bc(inputs["b_out"], OUT_C)
        in_maps.append(m)
    return in_maps


def kernel(**inputs):
    return run(inputs, FULL_CFG)



# revision 21
# speedup vs baseline: 1.0336x; 1.0336x over previous
"""ONGNN (2-layer ordered-neuron GNN) on 8 Trainium2 NeuronCores.

Strategy: shard DESTINATION nodes across the 8 cores (12500/core, padded to
12544 = 98*128).  Edges are bucketed on the host by (core, dst-window-of-128);
per-window message-tile counts are equalized across cores so one SPMD program
serves all cores.  Each conv layer:
  - AllGather of the bf16 node-feature shards -> full table in each core's DRAM
  - indirect-DMA gather of source rows (one 256B row per edge)
  - segment-sum via one-hot matmuls accumulated in PSUM (one-hot built on-chip
    from dst indices with a broadcast is_equal)
  - node-parallel dense math (transition matmul, softmax, cumsum, gating mix,
    layernorm) batched over superwindows of 4x128 nodes.
"""
import sys
import numpy as np

sys.path.insert(0, "/opt/trn_rl_repo")

import concourse.bass as bass
import concourse.bacc as bacc
import concourse.mybir as mybir
import concourse.tile as tile
from concourse import bass_utils

F = 128       # feature dim (IN_C == HID)
CH = 64       # CHUNK
OUT_C = 40
EPS = 1e-5
NCORES = 8

FULL_CFG = dict(N=100000, E=1000000, SH=12500, WPC=98, SW=4)
# SH: dst nodes per core; WPC: 128-node windows per core (ceil(SH/128));
# SW: windows per superwindow (batching factor for elementwise ops).

ABLATE = set()
STREAM_BUFS = 2
TINY_BUFS = 4
AOp = None  # filled lazily
AF = None


def _host_prep(x, edge_index, cfg):
    """Bucket edges by (core, window, src-chunk), build device arrays and the
    shared tile schedule.  Message stream order per superwindow: for each
    source-table chunk, for each window in the superwindow, that (w,c) run's
    tiles (padded to 128).  One dma_gather call covers one (sw, chunk) run."""
    N, E, SH, WPC, SW = cfg["N"], cfg["E"], cfg["SH"], cfg["WPC"], cfg["SW"]
    SHP = WPC * 128
    NFULL = NCORES * SHP
    n_sw = -(-WPC // SW)
    # source-table groups: window-aligned slices of each core's shard; the
    # AllGather is split into one collective per group so it can start as
    # soon as the producing superwindows finish.  8*rows per group must be
    # int16-addressable.  Last group smallest to shorten the serial tail.
    gsw = [(0, 7), (7, 14), (14, 21), (21, n_sw)]
    G_ENDS_W = [min(ge * SW, WPC) for _, ge in gsw]           # [28,56,84,98]
    row_ends = [w * 128 for w in G_ENDS_W]
    row_offs = [0] + row_ends[:-1]
    GROWS = [8 * (e - o) for o, e in zip(row_offs, row_ends)]  # rows per buf
    GOFF = np.concatenate([[0], np.cumsum(GROWS)])[:-1].tolist()
    NCH = len(GROWS)
    assert max(GROWS) <= 32767
    src = np.asarray(edge_index[0], dtype=np.int64)
    dst = np.asarray(edge_index[1], dtype=np.int64)

    core = dst // SH
    dst_loc = dst - core * SH
    win = dst_loc >> 7
    src_core = src // SH
    src_loc = src % SH
    chunk = np.searchsorted(np.asarray(row_ends), src_loc, side="right")
    # row inside the group's 8-core gather buffer
    grow = (src_core * np.asarray([e - o for o, e in zip(row_offs, row_ends)])[chunk]
            + (src_loc - np.asarray(row_offs)[chunk]))
    bucket = ((core * WPC + win) * NCH + chunk).astype(np.int64)
    order = np.argsort(bucket, kind="stable")
    bcnt = np.bincount(bucket, minlength=NCORES * WPC * NCH) \
        .reshape(NCORES, WPC, NCH)
    tpwc = -(-bcnt // 128)
    tpwc = tpwc.max(axis=0)                      # [WPC, NCH]
    for w in range(WPC):
        if tpwc[w].sum() == 0:
            tpwc[w, 0] = 1

    # schedule: per sw, per chunk, the window runs; global tile offsets
    sched = []
    t_acc = 0
    tile_win = []                                # window id of every tile
    for sw in range(n_sw):
        w0, w1 = sw * SW, min((sw + 1) * SW, WPC)
        runs = []
        for c in range(NCH):
            wruns = [(w, int(tpwc[w, c])) for w in range(w0, w1)
                     if tpwc[w, c] > 0]
            nt = sum(n for _, n in wruns)
            if nt == 0:
                continue
            runs.append((c, t_acc, wruns))
            for w, n in wruns:
                tile_win.extend([w] * n)
            t_acc += nt
        sched.append(runs)
    T = t_acc

    starts = np.zeros(NCORES * WPC * NCH + 1, np.int64)
    np.cumsum(bcnt.reshape(-1), out=starts[1:])
    idx16 = np.zeros((NCORES, T * 128), np.int16)
    rel = np.full((NCORES, T * 128), 512.0, np.float32)
    for k in range(NCORES):
        for sw in range(n_sw):
            for c, t0, wruns in sched[sw]:
                pos = t0 * 128
                for w, ntile in wruns:
                    b = (k * WPC + w) * NCH + c
                    sel = order[starts[b]:starts[b + 1]]
                    sel = sel[np.argsort(grow[sel], kind="stable")]
                    n = sel.size
                    idx16[k, pos:pos + n] = grow[sel].astype(np.int16)
                    rel[k, pos:pos + n] = (dst_loc[sel] - w * 128) \
                        .astype(np.float32)
                    pos += ntile * 128
    # device layouts
    idx16_dev = np.ascontiguousarray(idx16.reshape(NCORES, T * 8, 16)
                                     .transpose(0, 2, 1))     # [NC,16,T*8]
    idx16_dev = np.tile(idx16_dev, (1, 8, 1))                 # [NC,128,T*8]
    rel_dev = np.ascontiguousarray(rel.reshape(NCORES, T, 128)
                                   .transpose(0, 2, 1))       # [NC,128,T]
    # each rel value stored twice so the on-chip is_equal can use a
    # contiguous pair view (DVE 2x mode needs last-dim stride 1)
    rel_dev = np.repeat(rel_dev, 2, axis=2)                   # [NC,128,2T]

    cnt = np.bincount(dst, minlength=N).astype(np.float32)
    inv = 1.0 / np.maximum(cnt, 1.0)
    inv_dev = np.ones((NCORES, SHP), np.float32)
    for k in range(NCORES):
        inv_dev[k, :SH] = inv[k * SH:(k + 1) * SH]
    inv_dev = inv_dev.reshape(NCORES, WPC, 128).transpose(0, 2, 1)
    inv_dev = np.ascontiguousarray(inv_dev)

    xT_dev = np.zeros((NCORES, F, SHP), np.float32)
    x = np.asarray(x, np.float32)
    for k in range(NCORES):
        xT_dev[k, :, :SH] = x[k * SH:(k + 1) * SH].T

    return dict(idx16=idx16_dev, rel=rel_dev, inv=inv_dev, xT=xT_dev,
                sched=sched, tile_win=tile_win, T=T, SHP=SHP,
                GROWS=GROWS, GOFF=GOFF, row_offs=row_offs, row_ends=row_ends,
                gsw=gsw, tpwc=tpwc)


def _affine_trivial(g, b):
    return bool(np.allclose(g, 1.0, atol=1e-7) and np.allclose(b, 0.0, atol=1e-7))


def build_nc(cfg, prep, flags, debug_dumps=False, no_collectives=False, repeat=1):
    """Build the SPMD Bass program. flags: dict of *_trivial booleans."""
    global AOp, AF
    AOp = mybir.AluOpType
    AF = mybir.ActivationFunctionType
    dt = mybir.dt

    WPC, SW = cfg["WPC"], cfg["SW"]
    SHP = WPC * 128
    T = prep["T"]
    sched, tile_win = prep["sched"], prep["tile_win"]
    GROWS, GOFF = prep["GROWS"], prep["GOFF"]
    row_offs, row_ends, gsw = prep["row_offs"], prep["row_ends"], prep["gsw"]
    NFULL = NCORES * SHP

    nc = bacc.Bacc("TRN2", target_bir_lowering=False, debug=False,
                   num_devices=NCORES, num_swdge_queues=4)

    # ---- I/O ----
    xT_d = nc.dram_tensor("xT", [F, SHP], dt.bfloat16, kind="ExternalInput")
    win_d = nc.dram_tensor("w_in", [F, F], dt.bfloat16, kind="ExternalInput")
    idx_d = nc.dram_tensor("idx16", [128, T * 8], dt.int16, kind="ExternalInput")
    rel_d = nc.dram_tensor("rel", [128, 2 * T], dt.bfloat16, kind="ExternalInput")
    inv_d = nc.dram_tensor("inv", [128, WPC], dt.float32, kind="ExternalInput")
    tmw0_d = nc.dram_tensor("tmw0", [2 * F, CH], dt.bfloat16, kind="ExternalInput")
    tmw1_d = nc.dram_tensor("tmw1", [2 * F, CH], dt.bfloat16, kind="ExternalInput")
    wout_d = nc.dram_tensor("wout", [F, OUT_C], dt.bfloat16, kind="ExternalInput")
    iota_d = nc.dram_tensor("iota", [128, 128], dt.bfloat16, kind="ExternalInput")
    ident_d = nc.dram_tensor("ident", [128, 128], dt.bfloat16, kind="ExternalInput")
    out_d = nc.dram_tensor("out", [SHP, OUT_C], dt.float32, kind="ExternalOutput")
    gb_d = {}
    for nm in ("bin", "gin", "bein", "lng0", "lnb0", "lng1", "lnb1", "tmb0",
               "tmb1", "bout"):
        if not flags[nm + "_triv"]:
            width = {"tmb0": CH, "tmb1": CH, "bout": OUT_C}.get(nm, F)
            gb_d[nm] = nc.dram_tensor(nm, [128, width], dt.float32,
                                      kind="ExternalInput")

    dbg = {}
    if debug_dumps:
        for nm, shape, d in (("dbg_h0", [SHP, F], dt.float32),
                             ("dbg_hf0", [NFULL, F], dt.bfloat16),
                             ("dbg_m0", [SHP, F], dt.float32),
                             ("dbg_sig0", [128, WPC * CH], dt.float32),
                             ("dbg_g0", [128, 0], dt.bfloat16),
                             ("dbg_oh0", [128, 0], dt.bfloat16),
                             ("dbg_h1", [SHP, F], dt.float32)):
            if nm in ("dbg_g0", "dbg_oh0"):
                shape = [128, T * 128]
            dbg[nm] = nc.dram_tensor(nm, shape, d, kind="ExternalOutput")
    bounce0 = nc.dram_tensor("bounce0", [SHP, F], dt.bfloat16)
    bounce1 = nc.dram_tensor("bounce1", [SHP, F], dt.bfloat16)
    hfull0 = nc.dram_tensor("hfull0", [NFULL, F], dt.bfloat16, addr_space="Shared")
    hfull1 = nc.dram_tensor("hfull1", [NFULL, F], dt.bfloat16, addr_space="Shared")

    n_sw = (WPC + SW - 1) // SW

    with tile.TileContext(nc) as tc:
        import contextlib
        ctx = contextlib.ExitStack()
        with ctx:
            ctx.enter_context(nc.allow_low_precision(
                reason="bf16 elementwise; LN stats accumulate in fp32"))
            res = ctx.enter_context(tc.tile_pool(name="res", bufs=1))
            stream = ctx.enter_context(tc.tile_pool(name="stream", bufs=STREAM_BUFS))
            tiny = ctx.enter_context(tc.tile_pool(name="tiny", bufs=TINY_BUFS))
            small = ctx.enter_context(tc.tile_pool(name="small", bufs=3))
            psum = ctx.enter_context(tc.tile_pool(name="psum", bufs=2, space="PSUM"))
            pst = ctx.enter_context(tc.tile_pool(name="pst", bufs=2, space="PSUM"))

            # ---- residents / constants ----
            hT = res.tile([128, SHP], dt.bfloat16, tag="hT")       # feat-major own shard
            h_own = res.tile([128, SHP], dt.bfloat16, tag="h_own") # node-major own shard
            sigc = res.tile([128, WPC * CH], dt.bfloat16, tag="sigc")
            rel_t = res.tile([128, 2 * T], dt.bfloat16, tag="rel")
            inv_t = res.tile([128, WPC], dt.float32, tag="inv")
            w_in = res.tile([F, F], dt.bfloat16, tag="w_in")
            tmw = [[res.tile([F, CH], dt.bfloat16, tag=f"tmw{l}{h}", name=f"tmw{l}{h}")
                    for h in range(2)] for l in range(2)]
            wout = res.tile([F, OUT_C], dt.bfloat16, tag="wout")
            iota_t = res.tile([128, 128], dt.bfloat16, tag="iota")
            ident = res.tile([128, 128], dt.bfloat16, tag="ident")
            gb_t = {}
            for nm, d in gb_d.items():
                gb_t[nm] = res.tile(list(d.shape), dt.float32, tag=nm, name=nm)
                nc.sync.dma_start(gb_t[nm][:], d.ap())

            nc.sync.dma_start(rel_t[:], rel_d.ap())
            nc.sync.dma_start(inv_t[:], inv_d.ap())
            nc.sync.dma_start(w_in[:], win_d.ap())
            for l, d in enumerate((tmw0_d, tmw1_d)):
                nc.sync.dma_start(tmw[l][0][:], d.ap()[0:F, :])
                nc.sync.dma_start(tmw[l][1][:], d.ap()[F:2 * F, :])
            nc.sync.dma_start(wout[:], wout_d.ap())
            nc.sync.dma_start(iota_t[:], iota_d.ap())
            nc.sync.dma_start(ident[:], ident_d.ap())
            eps_t = res.tile([128, 1], dt.float32, tag="eps")
            nc.vector.memset(eps_t[:], EPS)

            def ln_smalls(su, sq, nw):
                """su,sq: [128,nw] fp32 -> (mean, rs) each [128,nw] fp32"""
                mean = small.tile([128, nw], dt.float32, tag="mean")
                nc.vector.tensor_scalar(mean[:], su, 1.0 / F, None, AOp.mult)
                t1 = small.tile([128, nw], dt.float32, tag="t1")
                nc.vector.tensor_tensor(t1[:], mean[:], su, AOp.mult)
                t2 = small.tile([128, nw], dt.float32, tag="t2")
                nc.vector.tensor_tensor(t2[:], sq, t1[:], AOp.subtract)
                srt = small.tile([128, nw], dt.float32, tag="srt")
                nc.scalar.activation(srt[:], t2[:], AF.Sqrt, bias=eps_t[:],
                                     scale=1.0 / F)
                rs = small.tile([128, nw], dt.float32, tag="rs")
                nc.vector.reciprocal(rs[:], srt[:])
                return mean, rs

            def apply_ln(dst_ap, u_ap, mean, rs, nw, gnm, bnm):
                """dst = (u - mean)*rs [* g] [+ b]; u_ap/dst [128, nw*128] bf16.
                Per-window fused tensor_scalar keeps operands contiguous
                (2x DVE mode); scalars are per-partition APs."""
                if gnm is None and bnm is None:
                    for j in range(nw):
                        nc.vector.tensor_scalar(
                            dst_ap[:, j * 128:(j + 1) * 128],
                            u_ap[:, j * 128:(j + 1) * 128],
                            mean[:, j:j + 1], rs[:, j:j + 1],
                            AOp.subtract, AOp.mult)
                    return
                u3 = u_ap.rearrange("p (w f) -> p w f", w=nw)
                d3 = dst_ap.rearrange("p (w f) -> p w f", w=nw)
                mb = mean[:].unsqueeze(2).broadcast_to([128, nw, 128])
                rb = rs[:].unsqueeze(2).broadcast_to([128, nw, 128])
                nc.vector.tensor_tensor(d3, u3, mb, AOp.subtract)
                nc.vector.tensor_tensor(d3, d3, rb, AOp.mult)
                if gnm is not None:
                    g3 = gb_t[gnm][:].unsqueeze(1).broadcast_to([128, nw, 128])
                    nc.vector.tensor_tensor(d3, d3, g3, AOp.mult)
                if bnm is not None:
                    b3 = gb_t[bnm][:].unsqueeze(1).broadcast_to([128, nw, 128])
                    nc.vector.tensor_tensor(d3, d3, b3, AOp.add)

            def _ag_group(bounce, hfull, g):
                r0, r1 = row_offs[g], row_ends[g]
                if no_collectives:
                    nc.sync.dma_start(hfull.ap()[GOFF[g]:GOFF[g] + (r1 - r0), :],
                                      bounce.ap()[r0:r1, :])
                else:
                    nc.gpsimd.collective_compute(
                        "AllGather", AOp.bypass,
                        replica_groups=[list(range(NCORES))],
                        ins=[bounce.ap()[r0:r1, :].opt()],
                        outs=[hfull.ap()[GOFF[g]:GOFF[g] + GROWS[g], :].opt()])

            def _once():
                # ============ Phase A: input MLP (h0 = LN(relu(x W + b))) ============
                for sw in range(n_sw):
                    w0 = sw * SW
                    nw = min(SW, WPC - w0)
                    nwf = nw * 128
                    ps_z = psum.tile([128, SW * 128], dt.float32, tag="ps_acc")
                    xt4 = stream.tile([128, SW * 128], dt.bfloat16, tag="m4", name="xt4")
                    nc.sync.dma_start(xt4[:, :nwf],
                                      xT_d.ap()[:, w0 * 128:w0 * 128 + nwf])
                    for j in range(nw):
                        nc.tensor.matmul(ps_z[:, j * 128:(j + 1) * 128],
                                         xt4[:, j * 128:(j + 1) * 128],
                                         w_in[:], start=True, stop=True)
                    # relu (+ optional b_in) -> r
                    r4 = stream.tile([128, SW * 128], dt.bfloat16, tag="u4", name="r4")
                    if "bin" in gb_t:
                        b3 = gb_t["bin"][:].unsqueeze(1).broadcast_to([128, nw, 128])
                        z3 = ps_z[:, :nwf].rearrange("p (w f) -> p w f", w=nw)
                        nc.vector.tensor_tensor(z3, z3, b3, AOp.add)
                    nc.scalar.activation(r4[:, :nwf], ps_z[:, :nwf], AF.Relu)
                    su = small.tile([128, SW], dt.float32, tag="su")
                    nc.vector.tensor_reduce(
                        su[:, :nw], r4[:, :nwf].rearrange("p (w f) -> p w f", w=nw),
                        mybir.AxisListType.X, AOp.add)
                    sqs = stream.tile([128, SW * 128], dt.bfloat16, tag="e4", name="sqs")
                    nc.scalar.activation(sqs[:, :nwf], r4[:, :nwf], AF.Square)
                    sq = small.tile([128, SW], dt.float32, tag="sq")
                    nc.vector.tensor_reduce(
                        sq[:, :nw], sqs[:, :nwf].rearrange("p (w f) -> p w f", w=nw),
                        mybir.AxisListType.X, AOp.add)
                    mean, rs = ln_smalls(su[:, :nw], sq[:, :nw], nw)
                    apply_ln(h_own[:, w0 * 128:w0 * 128 + nwf], r4[:, :nwf],
                             mean, rs, nw,
                             "gin" if "gin" in gb_t else None,
                             "bein" if "bein" in gb_t else None)
                    nc.sync.dma_start(
                        bounce0.ap()[w0 * 128:w0 * 128 + nwf, :]
                            .rearrange("(w p) f -> p w f", w=nw),
                        h_own[:, w0 * 128:w0 * 128 + nwf]
                            .rearrange("p (w f) -> p w f", w=nw))
                    ps_t4 = pst.tile([128, SW * 128], dt.bfloat16, tag="ps_t")
                    for j in range(nw):
                        w = w0 + j
                        nc.tensor.transpose(ps_t4[:, j * 128:(j + 1) * 128],
                                            h_own[:, w * 128:(w + 1) * 128], ident[:])
                    nc.scalar.activation(hT[:, w0 * 128:w0 * 128 + nwf],
                                         ps_t4[:, :nwf], AF.Copy)
                    for g, (gs, ge) in enumerate(gsw):
                        if sw == ge - 1:
                            _ag_group(bounce0, hfull0, g)

                if debug_dumps:
                    nc.sync.dma_start(
                        dbg["dbg_h0"].ap().rearrange("(w p) f -> p w f", w=WPC),
                        h_own[:].rearrange("p (w f) -> p w f", w=WPC))
                if debug_dumps:
                    nc.sync.dma_start(dbg["dbg_hf0"].ap(), hfull0.ap())

                # ============ Phase B/C: conv layers ============
                for l in range(2):
                    hfull = (hfull0, hfull1)[l]
                    for sw in range(n_sw):
                        w0 = sw * SW
                        nw = min(SW, WPC - w0)
                        nwf = nw * 128
                        runs = sched[sw]
                        c0 = runs[0][1]
                        c1 = runs[-1][1] + sum(n for _, n in runs[-1][2])
                        TS = c1 - c0
                        # gather (one dma_gather per source chunk) + onehot
                        g_t = stream.tile([128, TS * 128], dt.bfloat16, tag="g")
                        idx_sw = tiny.tile([128, TS * 8], dt.int16, tag="idx_sw")
                        nc.sync.dma_start(idx_sw[:], idx_d.ap()[:, c0 * 8:c1 * 8])
                        if "gather" in ABLATE:
                            nc.sync.dma_start(
                                g_t[:].rearrange("p (t f) -> p t f", t=TS),
                                hfull.ap()[0:TS * 128, :]
                                    .rearrange("(t p) f -> p t f", p=128))
                        for gi, (c, t0, wruns) in enumerate(
                                [] if "gather" in ABLATE else runs):
                            nt = sum(n for _, n in wruns)
                            nc.gpsimd.dma_gather(
                                g_t[:, (t0 - c0) * 128:(t0 - c0 + nt) * 128]
                                    .rearrange("p (t f) -> p t f", t=nt),
                                hfull.ap()[GOFF[c]:GOFF[c] + GROWS[c], :],
                                idx_sw[:, (t0 - c0) * 8:(t0 - c0 + nt) * 8],
                                nt * 128, nt * 128, F, single_packet=False,
                                queue_num=gi % 4)
                        oh_t = stream.tile([128, TS * 128], dt.bfloat16, tag="oh")
                        # pair views keep every last dim stride-1 so the DVE
                        # runs is_equal in 2x mode
                        oh4 = oh_t[:].rearrange("p (t f2 r) -> p t f2 r",
                                                t=TS, r=2)
                        iota_b = iota_t[:].rearrange("p (f2 r) -> p f2 r", r=2) \
                            .unsqueeze(1).broadcast_to([128, TS, 64, 2])
                        rel_b = rel_t[:, 2 * c0:2 * c1] \
                            .rearrange("p (t r) -> p t r", r=2) \
                            .unsqueeze(2).broadcast_to([128, TS, 64, 2])
                        if "onehot" not in ABLATE:
                            nc.vector.tensor_tensor(oh4, iota_b, rel_b, AOp.is_equal)
                        else:
                            nc.vector.memset(oh_t[:], 0.0)
                        if debug_dumps and l == 0:
                            nc.sync.dma_start(dbg["dbg_g0"].ap()[:, c0 * 128:c1 * 128],
                                              g_t[:])
                            nc.sync.dma_start(dbg["dbg_oh0"].ap()[:, c0 * 128:c1 * 128],
                                              oh_t[:])
                        # segment matmuls, window-major so each window's PSUM
                        # accumulation group opens and closes contiguously
                        ps_m = psum.tile([128, SW * 128], dt.float32, tag="ps_acc")
                        for j in range(nw):
                            w = w0 + j
                            wtiles = [t for t in range(c0, c1) if tile_win[t] == w]
                            if "seg" in ABLATE:
                                wtiles = wtiles[:1]
                            for i, t in enumerate(wtiles):
                                tl = t - c0
                                nc.tensor.matmul(
                                    ps_m[:, j * 128:(j + 1) * 128],
                                    oh_t[:, tl * 128:(tl + 1) * 128],
                                    g_t[:, tl * 128:(tl + 1) * 128],
                                    start=(i == 0), stop=(i == len(wtiles) - 1))
                        # m (scaled) node-major
                        m4 = stream.tile([128, SW * 128], dt.bfloat16, tag="m4")
                        m3 = m4[:, :nwf].rearrange("p (w f) -> p w f", w=nw)
                        iv = inv_t[:, w0:w0 + nw].unsqueeze(2).broadcast_to(
                            [128, nw, 128])
                        nc.vector.tensor_tensor(
                            m3, ps_m[:, :nwf].rearrange("p (w f) -> p w f", w=nw),
                            iv, AOp.mult)
                        if debug_dumps and l == 0:
                            nc.sync.dma_start(
                                dbg["dbg_m0"].ap()[w0 * 128:w0 * 128 + nwf, :]
                                    .rearrange("(w p) f -> p w f", w=nw),
                                m3)
                        # mT batched transpose + tm matmuls
                        ps_t4m = pst.tile([128, SW * 128], dt.bfloat16, tag="ps_t")
                        for j in range(nw):
                            nc.tensor.transpose(ps_t4m[:, j * 128:(j + 1) * 128],
                                                m4[:, j * 128:(j + 1) * 128],
                                                ident[:])
                        mt4 = tiny.tile([128, SW * 128], dt.bfloat16, tag="mt")
                        nc.scalar.activation(mt4[:, :nwf], ps_t4m[:, :nwf], AF.Copy)
                        ps_tm = psum.tile([128, SW * CH], dt.float32, tag="ps_sm")
                        for j in range(nw):
                            w = w0 + j
                            nc.tensor.matmul(ps_tm[:, j * CH:(j + 1) * CH],
                                             hT[:, w * 128:(w + 1) * 128],
                                             tmw[l][0][:], start=True, stop=False)
                            nc.tensor.matmul(ps_tm[:, j * CH:(j + 1) * CH],
                                             mt4[:, j * 128:(j + 1) * 128],
                                             tmw[l][1][:],
                                             start=False, stop=True)
                        nwc = nw * CH
                        if ("tmb0", "tmb1")[l] in gb_t:
                            tb = gb_t[("tmb0", "tmb1")[l]][:].unsqueeze(1) \
                                .broadcast_to([128, nw, CH])
                            z3 = ps_tm[:, :nwc].rearrange("p (w c) -> p w c", w=nw)
                            nc.vector.tensor_tensor(z3, z3, tb, AOp.add)
                        # softmax (no max-sub) + cumsum
                        e4 = stream.tile([128, SW * CH], dt.float32, tag="e4")
                        nc.scalar.activation(e4[:, :nwc], ps_tm[:, :nwc], AF.Exp)
                        s4 = small.tile([128, SW], dt.float32, tag="s4")
                        nc.vector.tensor_reduce(
                            s4[:, :nw], e4[:, :nwc].rearrange("p (w c) -> p w c", w=nw),
                            mybir.AxisListType.X, AOp.add)
                        r4s = small.tile([128, SW], dt.float32, tag="r4s")
                        nc.vector.reciprocal(r4s[:, :nw], s4[:, :nw])
                        cs4 = stream.tile([128, SW * CH], dt.float32, tag="cs4")
                        for j in range(nw):
                            nc.vector.tensor_tensor_scan(
                                cs4[:, j * CH:(j + 1) * CH],
                                e4[:, j * CH:(j + 1) * CH],
                                e4[:, j * CH:(j + 1) * CH], 0.0, AOp.add, AOp.bypass)
                        # sig update
                        rb = r4s[:, :nw].unsqueeze(2).broadcast_to([128, nw, CH])
                        cs3 = cs4[:, :nwc].rearrange("p (w c) -> p w c", w=nw)
                        sg_cols = sigc[:, w0 * CH:w0 * CH + nwc]
                        sg3 = sg_cols.rearrange("p (w c) -> p w c", w=nw)
                        if l == 0:
                            nc.vector.tensor_tensor(sg3, cs3, rb, AOp.mult)
                            sig_src = sg_cols
                        else:
                            t4 = stream.tile([128, SW * CH], dt.bfloat16, tag="t4")
                            t3 = t4[:, :nwc].rearrange("p (w c) -> p w c", w=nw)
                            nc.vector.tensor_tensor(t3, cs3, rb, AOp.mult)
                            a4 = stream.tile([128, SW * CH], dt.bfloat16, tag="a4")
                            nc.vector.tensor_tensor(a4[:, :nwc], sg_cols, t4[:, :nwc],
                                                    AOp.mult)
                            nc.vector.tensor_tensor(t4[:, :nwc], t4[:, :nwc],
                                                    a4[:, :nwc], AOp.subtract)
                            nc.vector.tensor_tensor(t4[:, :nwc], t4[:, :nwc],
                                                    sg_cols, AOp.add)
                            sig_src = t4[:, :nwc]
                        # mix u = h*sig + m*(1-sig) = m + sig*(h-m)
                        # sig broadcast over the repeat axis (64 -> 128)
                        sig_b = sig_src.rearrange("p (w c) -> p w c", w=nw) \
                            .unsqueeze(3).broadcast_to([128, nw, CH, 2])
                        hcols = h_own[:, w0 * 128:w0 * 128 + nwf]
                        u4 = stream.tile([128, SW * 128], dt.bfloat16, tag="u4")
                        nc.vector.tensor_tensor(u4[:, :nwf], hcols, m4[:, :nwf],
                                                AOp.subtract)
                        u4d = u4[:, :nwf].rearrange("p (w c r) -> p w c r",
                                                    w=nw, r=2)
                        nc.vector.tensor_tensor(u4d, u4d, sig_b, AOp.mult)
                        nc.vector.tensor_tensor(u4[:, :nwf], u4[:, :nwf], m4[:, :nwf],
                                                AOp.add)
                        # LN stats
                        su = small.tile([128, SW], dt.float32, tag="su")
                        nc.vector.tensor_reduce(
                            su[:, :nw], u4[:, :nwf].rearrange("p (w f) -> p w f", w=nw),
                            mybir.AxisListType.X, AOp.add)
                        sqs = stream.tile([128, SW * 128], dt.bfloat16, tag="e4", name="sqs")
                        nc.scalar.activation(sqs[:, :nwf], u4[:, :nwf], AF.Square)
                        sq = small.tile([128, SW], dt.float32, tag="sq")
                        nc.vector.tensor_reduce(
                            sq[:, :nw], sqs[:, :nwf].rearrange("p (w f) -> p w f", w=nw),
                            mybir.AxisListType.X, AOp.add)
                        mean, rs = ln_smalls(su[:, :nw], sq[:, :nw], nw)
                        gnm = ("lng0", "lng1")[l]
                        bnm = ("lnb0", "lnb1")[l]
                        if l == 0:
                            apply_ln(hcols, u4[:, :nwf], mean, rs, nw,
                                     gnm if gnm in gb_t else None,
                                     bnm if bnm in gb_t else None)
                            nc.sync.dma_start(
                                bounce1.ap()[w0 * 128:w0 * 128 + nwf, :]
                                    .rearrange("(w p) f -> p w f", w=nw),
                                hcols.rearrange("p (w f) -> p w f", w=nw))
                            ps_t4 = pst.tile([128, SW * 128], dt.bfloat16, tag="ps_t")
                            for j in range(nw):
                                w = w0 + j
                                nc.tensor.transpose(
                                    ps_t4[:, j * 128:(j + 1) * 128],
                                    h_own[:, w * 128:(w + 1) * 128], ident[:])
                            nc.scalar.activation(hT[:, w0 * 128:w0 * 128 + nwf],
                                                 ps_t4[:, :nwf], AF.Copy)
                        else:
                            h2 = stream.tile([128, SW * 128], dt.bfloat16, tag="hx", name="h2")
                            apply_ln(h2[:, :nwf], u4[:, :nwf], mean, rs, nw,
                                     gnm if gnm in gb_t else None,
                                     bnm if bnm in gb_t else None)
                            ob = stream.tile([128, SW * OUT_C], dt.float32, tag="ob")
                            ps_o = psum.tile([128, SW * OUT_C], dt.float32, tag="ps_sm")
                            ps_t4 = pst.tile([128, SW * 128], dt.bfloat16, tag="ps_t")
                            for j in range(nw):
                                nc.tensor.transpose(
                                    ps_t4[:, j * 128:(j + 1) * 128],
                                    h2[:, j * 128:(j + 1) * 128], ident[:])
                            h2t4 = tiny.tile([128, SW * 128], dt.bfloat16, tag="h2t")
                            nc.scalar.activation(h2t4[:, :nwf], ps_t4[:, :nwf],
                                                 AF.Copy)
                            for j in range(nw):
                                nc.tensor.matmul(ps_o[:, j * OUT_C:(j + 1) * OUT_C],
                                                 h2t4[:, j * 128:(j + 1) * 128],
                                                 wout[:], start=True, stop=True)
                            nwo = nw * OUT_C
                            if "bout" in gb_t:
                                bb = gb_t["bout"][:].unsqueeze(1).broadcast_to(
                                    [128, nw, OUT_C])
                                o3 = ob[:, :nwo].rearrange("p (w o) -> p w o", w=nw)
                                nc.vector.tensor_tensor(
                                    o3, ps_o[:, :nwo].rearrange("p (w o) -> p w o", w=nw),
                                    bb, AOp.add)
                            else:
                                nc.vector.tensor_copy(ob[:, :nwo], ps_o[:, :nwo])
                            nc.sync.dma_start(
                                out_d.ap()[w0 * 128:w0 * 128 + nwf, :]
                                    .rearrange("(w p) o -> p w o", w=nw),
                                ob[:, :nwo].rearrange("p (w o) -> p w o", w=nw))
                    if l == 0:
                        for g in range(len(gsw)):
                            _ag_group(bounce1, hfull1, g)
                    if l == 0 and debug_dumps:
                        nc.sync.dma_start(dbg["dbg_sig0"].ap(), sigc[:])
                        nc.sync.dma_start(
                            dbg["dbg_h1"].ap().rearrange("(w p) f -> p w f", w=WPC),
                            h_own[:].rearrange("p (w f) -> p w f", w=WPC))


            for _rep in range(repeat):
                _once()

    nc.compile()
    return nc


_CACHE = {}


def _sched_key(prep):
    return (tuple(prep["tile_win"]),
            tuple((c, t0, tuple(wr)) for sw in prep["sched"]
                  for c, t0, wr in sw))


def _get_compiled(cfg, prep, flags):
    key = (_sched_key(prep), tuple(sorted(flags.items())))
    if key not in _CACHE:
        _CACHE[key] = build_nc(cfg, prep, flags)
    return _CACHE[key]


class PjrtRunner:
    """Persistent jitted shard_map executor for one compiled nc (8 cores)."""

    def __init__(self, nc, donate=True):
        import jax
        from jax.experimental.shard_map import shard_map
        from jax.sharding import Mesh, PartitionSpec
        from concourse import bass2jax

        bass2jax.install_neuronx_cc_hook()
        self.nc = nc
        in_names, out_names, out_avals, zero_outs = [], [], [], []
        partition_name = (nc.partition_id_tensor.name
                          if nc.partition_id_tensor else None)
        for alloc in nc.m.functions[0].allocations:
            if not isinstance(alloc, mybir.MemoryLocationSet):
                continue
            name = alloc.memorylocations[0].name
            if alloc.kind == "ExternalInput":
                if name != partition_name:
                    in_names.append(name)
            elif alloc.kind == "ExternalOutput":
                import jax.core as jcore
                out_names.append(name)
                aval = jax.core.ShapedArray(
                    tuple(alloc.tensor_shape), mybir.dt.np(alloc.dtype))
                out_avals.append(aval)
                zero_outs.append(np.zeros(alloc.tensor_shape,
                                          mybir.dt.np(alloc.dtype)))
        self.n_params = len(in_names)
        self.out_names = list(out_names)
        self.zero_outs = zero_outs
        all_in = in_names + out_names
        if partition_name is not None:
            all_in.append(partition_name)
        self.in_names_data = in_names

        def _body(*args):
            operands = list(args)
            if partition_name is not None:
                operands.append(bass2jax.partition_id_tensor())
            outs = bass2jax._bass_exec_p.bind(
                *operands,
                out_avals=tuple(out_avals),
                in_names=tuple(all_in),
                out_names=tuple(out_names),
                lowering_input_output_aliases=(),
                sim_require_finite=True,
                sim_require_nnan=True,
                nc=nc,
            )
            return tuple(outs)

        devices = jax.devices()[:NCORES]
        self.mesh = Mesh(np.asarray(devices), ("core",))
        n_out = len(out_names)
        donate_nums = (tuple(range(self.n_params, self.n_params + n_out))
                       if donate else ())
        in_specs = (PartitionSpec("core"),) * (self.n_params + n_out)
        out_specs = (PartitionSpec("core"),) * n_out
        self.fn = jax.jit(
            shard_map(_body, mesh=self.mesh, in_specs=in_specs,
                      out_specs=out_specs, check_rep=False),
            donate_argnums=donate_nums, keep_unused=True)

    def concat_inputs(self, in_maps):
        return [
            np.concatenate([np.asarray(in_maps[c][nm]) for c in range(NCORES)],
                           axis=0)
            for nm in self.in_names_data
        ]

    def zeros(self):
        return [np.zeros((NCORES * z.shape[0], *z.shape[1:]), z.dtype)
                for z in self.zero_outs]

    def __call__(self, concat_in, zeros):
        import jax
        outs = self.fn(*concat_in, *zeros)
        return {nm: np.asarray(outs[i]) for i, nm in enumerate(self.out_names)}


_RUNNERS = {}


def get_runner(cfg, prep, flags):
    key = (_sched_key(prep), tuple(sorted(flags.items())))
    if key not in _RUNNERS:
        _RUNNERS[key] = PjrtRunner(_get_compiled(cfg, prep, flags))
    return _RUNNERS[key]


def run(inputs, cfg):
    x = np.asarray(inputs["x"], np.float32)
    prep = _host_prep(x, np.asarray(inputs["edge_index"]), cfg)
    SH = cfg["SH"]

    flags = make_flags(inputs)
    runner = get_runner(cfg, prep, flags)
    in_maps = make_in_maps(inputs, prep, flags)
    out = runner(runner.concat_inputs(in_maps), runner.zeros())["out"]
    SHP = prep["SHP"]
    out = out.reshape(NCORES, SHP, OUT_C)[:, :SH, :]
    return np.ascontiguousarray(out.reshape(NCORES * SH, OUT_C), dtype=np.float32)


def make_flags(inputs):
    return {
        "bin_triv": _affine_trivial(1.0, inputs["b_in"]),
        "gin_triv": _affine_trivial(inputs["g_in"], 0.0),
        "bein_triv": _affine_trivial(1.0, inputs["be_in"]),
        "lng0_triv": _affine_trivial(inputs["ln_g0"], 0.0),
        "lnb0_triv": _affine_trivial(1.0, inputs["ln_b0"]),
        "lng1_triv": _affine_trivial(inputs["ln_g1"], 0.0),
        "lnb1_triv": _affine_trivial(1.0, inputs["ln_b1"]),
        "tmb0_triv": _affine_trivial(1.0, inputs["tm_b0"]),
        "tmb1_triv": _affine_trivial(1.0, inputs["tm_b1"]),
        "bout_triv": _affine_trivial(1.0, inputs["b_out"]),
    }


def make_in_maps(inputs, prep, flags):
    def bc(v, width):
        return np.tile(np.asarray(v, np.float32).reshape(1, width), (128, 1))

    in_maps = []
    for k in range(NCORES):
        import ml_dtypes
        bf16 = ml_dtypes.bfloat16
        m = {
            "xT": prep["xT"][k].astype(bf16),
            "w_in": np.asarray(inputs["W_in"], np.float32).astype(bf16),
            "idx16": prep["idx16"][k],
            "rel": prep["rel"][k].astype(bf16),
            "inv": prep["inv"][k],
            "tmw0": np.asarray(inputs["tm_W0"], np.float32).astype(bf16),
            "tmw1": np.asarray(inputs["tm_W1"], np.float32).astype(bf16),
            "wout": np.asarray(inputs["W_out"], np.float32).astype(bf16),
            "iota": np.tile(np.arange(128, dtype=np.float32)[None, :],
                            (128, 1)).astype(bf16),
            "ident": np.eye(128, dtype=np.float32).astype(bf16),
        }
        if not flags["bin_triv"]:
            m["bin"] = bc(inputs["b_in"], F)
        if not flags["gin_triv"]:
            m["gin"] = bc(inputs["g_in"], F)
        if not flags["bein_triv"]:
            m["bein"] = bc(inputs["be_in"], F)
        for nm, src in (("lng0", "ln_g0"), ("lnb0", "ln_b0"),
                        ("lng1", "ln_g1"), ("lnb1", "ln_b1")):
            if not flags[nm + "_triv"]:
                m[nm] = bc(inputs[src], F)
        if not flags["tmb0_triv"]:
            m["tmb0"] = bc(inputs["tm_b0"], CH)
        if not flags["tmb1_triv"]:
            m["tmb1"] = bc(inputs["tm_b1"], CH)
        if not flags["bout_triv"]:
            m["bout"] = bc(inputs["b_out"], OUT_C)
        in_maps.append(m)
    return in_maps


def kernel(**inputs):
    return run(inputs, FULL_CFG)



# revision 22
# speedup vs baseline: 1.5708x; 1.5198x over previous
"""ONGNN (2-layer ordered-neuron GNN) on 8 Trainium2 NeuronCores.

Strategy: shard DESTINATION nodes across the 8 cores (12500/core, padded to
12544 = 98*128).  Edges are bucketed on the host by (core, dst-window-of-128);
per-window message-tile counts are equalized across cores so one SPMD program
serves all cores.  Each conv layer:
  - AllGather of the bf16 node-feature shards -> full table in each core's DRAM
  - indirect-DMA gather of source rows (one 256B row per edge)
  - segment-sum via one-hot matmuls accumulated in PSUM (one-hot built on-chip
    from dst indices with a broadcast is_equal)
  - node-parallel dense math (transition matmul, softmax, cumsum, gating mix,
    layernorm) batched over superwindows of 4x128 nodes.
"""
import sys
import numpy as np

sys.path.insert(0, "/opt/trn_rl_repo")

import concourse.bass as bass
import concourse.bacc as bacc
import concourse.mybir as mybir
import concourse.tile as tile
from concourse import bass_utils

F = 128       # feature dim (IN_C == HID)
CH = 64       # CHUNK
OUT_C = 40
EPS = 1e-5
NCORES = 8

FULL_CFG = dict(N=100000, E=1000000, SH=12500, WPC=98, SW=4)
# SH: dst nodes per core; WPC: 128-node windows per core (ceil(SH/128));
# SW: windows per superwindow (batching factor for elementwise ops).

ABLATE = set()
STREAM_BUFS = 2
TINY_BUFS = 4
AOp = None  # filled lazily
AF = None


def _host_prep(x, edge_index, cfg):
    """Bucket edges by (core, window, src-chunk), build device arrays and the
    shared tile schedule.  Message stream order per superwindow: for each
    source-table chunk, for each window in the superwindow, that (w,c) run's
    tiles (padded to 128).  One dma_gather call covers one (sw, chunk) run."""
    N, E, SH, WPC, SW = cfg["N"], cfg["E"], cfg["SH"], cfg["WPC"], cfg["SW"]
    SHP = WPC * 128
    NFULL = NCORES * SHP
    CROWS = max(SHP, (32767 // SHP) * SHP)      # chunk rows (int16-addressable)
    NCH = -(-NFULL // CROWS)
    n_sw = -(-WPC // SW)
    src = np.asarray(edge_index[0], dtype=np.int64)
    dst = np.asarray(edge_index[1], dtype=np.int64)

    core = dst // SH
    dst_loc = dst - core * SH
    win = dst_loc >> 7
    grow = (src // SH) * SHP + (src % SH)       # padded full-table row
    chunk = grow // CROWS
    bucket = ((core * WPC + win) * NCH + chunk).astype(np.int64)
    order = np.argsort(bucket, kind="stable")
    bcnt = np.bincount(bucket, minlength=NCORES * WPC * NCH) \
        .reshape(NCORES, WPC, NCH)
    tpwc = -(-bcnt // 128)
    tpwc = tpwc.max(axis=0)                      # [WPC, NCH]
    for w in range(WPC):
        if tpwc[w].sum() == 0:
            tpwc[w, 0] = 1

    # schedule: per sw, per chunk, the window runs; global tile offsets
    sched = []
    t_acc = 0
    tile_win = []                                # window id of every tile
    for sw in range(n_sw):
        w0, w1 = sw * SW, min((sw + 1) * SW, WPC)
        runs = []
        for c in range(NCH):
            wruns = [(w, int(tpwc[w, c])) for w in range(w0, w1)
                     if tpwc[w, c] > 0]
            nt = sum(n for _, n in wruns)
            if nt == 0:
                continue
            runs.append((c, t_acc, wruns))
            for w, n in wruns:
                tile_win.extend([w] * n)
            t_acc += nt
        sched.append(runs)
    T = t_acc

    starts = np.zeros(NCORES * WPC * NCH + 1, np.int64)
    np.cumsum(bcnt.reshape(-1), out=starts[1:])
    idx16 = np.zeros((NCORES, T * 128), np.int16)
    rel = np.full((NCORES, T * 128), 512.0, np.float32)
    for k in range(NCORES):
        for sw in range(n_sw):
            for c, t0, wruns in sched[sw]:
                pos = t0 * 128
                for w, ntile in wruns:
                    b = (k * WPC + w) * NCH + c
                    sel = order[starts[b]:starts[b + 1]]
                    sel = sel[np.argsort(grow[sel], kind="stable")]
                    n = sel.size
                    idx16[k, pos:pos + n] = (grow[sel] - c * CROWS) \
                        .astype(np.int16)
                    rel[k, pos:pos + n] = (dst_loc[sel] - w * 128) \
                        .astype(np.float32)
                    pos += ntile * 128
    # device layouts
    idx16_dev = np.ascontiguousarray(idx16.reshape(NCORES, T * 8, 16)
                                     .transpose(0, 2, 1))     # [NC,16,T*8]
    idx16_dev = np.tile(idx16_dev, (1, 8, 1))                 # [NC,128,T*8]
    rel_dev = np.ascontiguousarray(rel.reshape(NCORES, T, 128)
                                   .transpose(0, 2, 1))       # [NC,128,T]
    # each rel value stored twice so the on-chip is_equal can use a
    # contiguous pair view (DVE 2x mode needs last-dim stride 1)
    rel_dev = np.repeat(rel_dev, 2, axis=2)                   # [NC,128,2T]

    cnt = np.bincount(dst, minlength=N).astype(np.float32)
    inv = 1.0 / np.maximum(cnt, 1.0)
    inv_dev = np.ones((NCORES, SHP), np.float32)
    for k in range(NCORES):
        inv_dev[k, :SH] = inv[k * SH:(k + 1) * SH]
    inv_dev = inv_dev.reshape(NCORES, WPC, 128).transpose(0, 2, 1)
    inv_dev = np.ascontiguousarray(inv_dev)

    xT_dev = np.zeros((NCORES, F, SHP), np.float32)
    x = np.asarray(x, np.float32)
    for k in range(NCORES):
        xT_dev[k, :, :SH] = x[k * SH:(k + 1) * SH].T

    return dict(idx16=idx16_dev, rel=rel_dev, inv=inv_dev, xT=xT_dev,
                sched=sched, tile_win=tile_win, T=T, SHP=SHP, CROWS=CROWS,
                tpwc=tpwc)


def _affine_trivial(g, b):
    return bool(np.allclose(g, 1.0, atol=1e-7) and np.allclose(b, 0.0, atol=1e-7))


def build_nc(cfg, prep, flags, debug_dumps=False, no_collectives=False, repeat=1):
    """Build the SPMD Bass program. flags: dict of *_trivial booleans."""
    global AOp, AF
    AOp = mybir.AluOpType
    AF = mybir.ActivationFunctionType
    dt = mybir.dt

    WPC, SW = cfg["WPC"], cfg["SW"]
    SHP = WPC * 128
    T = prep["T"]
    sched, tile_win, CROWS = prep["sched"], prep["tile_win"], prep["CROWS"]
    NFULL = NCORES * SHP

    nc = bacc.Bacc("TRN2", target_bir_lowering=False, debug=False,
                   num_devices=NCORES, num_swdge_queues=4)

    # ---- I/O ----
    xT_d = nc.dram_tensor("xT", [F, SHP], dt.bfloat16, kind="ExternalInput")
    win_d = nc.dram_tensor("w_in", [F, F], dt.bfloat16, kind="ExternalInput")
    idx_d = nc.dram_tensor("idx16", [128, T * 8], dt.int16, kind="ExternalInput")
    rel_d = nc.dram_tensor("rel", [128, 2 * T], dt.bfloat16, kind="ExternalInput")
    inv_d = nc.dram_tensor("inv", [128, WPC], dt.float32, kind="ExternalInput")
    tmw0_d = nc.dram_tensor("tmw0", [2 * F, CH], dt.bfloat16, kind="ExternalInput")
    tmw1_d = nc.dram_tensor("tmw1", [2 * F, CH], dt.bfloat16, kind="ExternalInput")
    wout_d = nc.dram_tensor("wout", [F, OUT_C], dt.bfloat16, kind="ExternalInput")
    iota_d = nc.dram_tensor("iota", [128, 128], dt.bfloat16, kind="ExternalInput")
    ident_d = nc.dram_tensor("ident", [128, 128], dt.bfloat16, kind="ExternalInput")
    out_d = nc.dram_tensor("out", [SHP, OUT_C], dt.float32, kind="ExternalOutput")
    gb_d = {}
    for nm in ("bin", "gin", "bein", "lng0", "lnb0", "lng1", "lnb1", "tmb0",
               "tmb1", "bout"):
        if not flags[nm + "_triv"]:
            width = {"tmb0": CH, "tmb1": CH, "bout": OUT_C}.get(nm, F)
            gb_d[nm] = nc.dram_tensor(nm, [128, width], dt.float32,
                                      kind="ExternalInput")

    dbg = {}
    if debug_dumps:
        for nm, shape, d in (("dbg_h0", [SHP, F], dt.float32),
                             ("dbg_hf0", [NFULL, F], dt.bfloat16),
                             ("dbg_m0", [SHP, F], dt.float32),
                             ("dbg_sig0", [128, WPC * CH], dt.float32),
                             ("dbg_g0", [128, 0], dt.bfloat16),
                             ("dbg_oh0", [128, 0], dt.bfloat16),
                             ("dbg_h1", [SHP, F], dt.float32)):
            if nm in ("dbg_g0", "dbg_oh0"):
                shape = [128, T * 128]
            dbg[nm] = nc.dram_tensor(nm, shape, d, kind="ExternalOutput")
    bounce0 = nc.dram_tensor("bounce0", [SHP, F], dt.bfloat16)
    bounce1 = nc.dram_tensor("bounce1", [SHP, F], dt.bfloat16)
    hfull0 = nc.dram_tensor("hfull0", [NFULL, F], dt.bfloat16, addr_space="Shared")
    hfull1 = nc.dram_tensor("hfull1", [NFULL, F], dt.bfloat16, addr_space="Shared")

    n_sw = (WPC + SW - 1) // SW

    with tile.TileContext(nc) as tc:
        import contextlib
        ctx = contextlib.ExitStack()
        with ctx:
            ctx.enter_context(nc.allow_low_precision(
                reason="bf16 elementwise; LN stats accumulate in fp32"))
            res = ctx.enter_context(tc.tile_pool(name="res", bufs=1))
            stream = ctx.enter_context(tc.tile_pool(name="stream", bufs=STREAM_BUFS))
            tiny = ctx.enter_context(tc.tile_pool(name="tiny", bufs=TINY_BUFS))
            small = ctx.enter_context(tc.tile_pool(name="small", bufs=3))
            psum = ctx.enter_context(tc.tile_pool(name="psum", bufs=2, space="PSUM"))
            pst = ctx.enter_context(tc.tile_pool(name="pst", bufs=2, space="PSUM"))

            # ---- residents / constants ----
            hT = res.tile([128, SHP], dt.bfloat16, tag="hT")       # feat-major own shard
            h_own = res.tile([128, SHP], dt.bfloat16, tag="h_own") # node-major own shard
            sigc = res.tile([128, WPC * CH], dt.bfloat16, tag="sigc")
            rel_t = res.tile([128, 2 * T], dt.bfloat16, tag="rel")
            inv_t = res.tile([128, WPC], dt.float32, tag="inv")
            w_in = res.tile([F, F], dt.bfloat16, tag="w_in")
            tmw = [[res.tile([F, CH], dt.bfloat16, tag=f"tmw{l}{h}", name=f"tmw{l}{h}")
                    for h in range(2)] for l in range(2)]
            wout = res.tile([F, OUT_C], dt.bfloat16, tag="wout")
            iota_t = res.tile([128, 128], dt.bfloat16, tag="iota")
            ident = res.tile([128, 128], dt.bfloat16, tag="ident")
            gb_t = {}
            for nm, d in gb_d.items():
                gb_t[nm] = res.tile(list(d.shape), dt.float32, tag=nm, name=nm)
                nc.sync.dma_start(gb_t[nm][:], d.ap())

            nc.sync.dma_start(rel_t[:], rel_d.ap())
            nc.sync.dma_start(inv_t[:], inv_d.ap())
            nc.sync.dma_start(w_in[:], win_d.ap())
            for l, d in enumerate((tmw0_d, tmw1_d)):
                nc.sync.dma_start(tmw[l][0][:], d.ap()[0:F, :])
                nc.sync.dma_start(tmw[l][1][:], d.ap()[F:2 * F, :])
            nc.sync.dma_start(wout[:], wout_d.ap())
            nc.sync.dma_start(iota_t[:], iota_d.ap())
            nc.sync.dma_start(ident[:], ident_d.ap())
            eps_t = res.tile([128, 1], dt.float32, tag="eps")
            nc.vector.memset(eps_t[:], EPS)

            def ln_smalls(su, sq, nw):
                """su,sq: [128,nw] fp32 -> (mean, rs) each [128,nw] fp32"""
                mean = small.tile([128, nw], dt.float32, tag="mean")
                nc.vector.tensor_scalar(mean[:], su, 1.0 / F, None, AOp.mult)
                t1 = small.tile([128, nw], dt.float32, tag="t1")
                nc.vector.tensor_tensor(t1[:], mean[:], su, AOp.mult)
                t2 = small.tile([128, nw], dt.float32, tag="t2")
                nc.vector.tensor_tensor(t2[:], sq, t1[:], AOp.subtract)
                srt = small.tile([128, nw], dt.float32, tag="srt")
                nc.scalar.activation(srt[:], t2[:], AF.Sqrt, bias=eps_t[:],
                                     scale=1.0 / F)
                rs = small.tile([128, nw], dt.float32, tag="rs")
                nc.vector.reciprocal(rs[:], srt[:])
                return mean, rs

            def apply_ln(dst_ap, u_ap, mean, rs, nw, gnm, bnm):
                """dst = (u - mean)*rs [* g] [+ b]; u_ap/dst [128, nw*128] bf16.
                Per-window fused tensor_scalar keeps operands contiguous
                (2x DVE mode); scalars are per-partition APs."""
                if gnm is None and bnm is None:
                    for j in range(nw):
                        nc.vector.tensor_scalar(
                            dst_ap[:, j * 128:(j + 1) * 128],
                            u_ap[:, j * 128:(j + 1) * 128],
                            mean[:, j:j + 1], rs[:, j:j + 1],
                            AOp.subtract, AOp.mult)
                    return
                u3 = u_ap.rearrange("p (w f) -> p w f", w=nw)
                d3 = dst_ap.rearrange("p (w f) -> p w f", w=nw)
                mb = mean[:].unsqueeze(2).broadcast_to([128, nw, 128])
                rb = rs[:].unsqueeze(2).broadcast_to([128, nw, 128])
                nc.vector.tensor_tensor(d3, u3, mb, AOp.subtract)
                nc.vector.tensor_tensor(d3, d3, rb, AOp.mult)
                if gnm is not None:
                    g3 = gb_t[gnm][:].unsqueeze(1).broadcast_to([128, nw, 128])
                    nc.vector.tensor_tensor(d3, d3, g3, AOp.mult)
                if bnm is not None:
                    b3 = gb_t[bnm][:].unsqueeze(1).broadcast_to([128, nw, 128])
                    nc.vector.tensor_tensor(d3, d3, b3, AOp.add)

            def _ag(bounce, hfull):
                if no_collectives:
                    nc.sync.dma_start(hfull.ap()[0:SHP, :], bounce.ap())
                else:
                    nc.gpsimd.collective_compute(
                        "AllGather", AOp.bypass,
                        replica_groups=[list(range(NCORES))],
                        ins=[bounce.ap().opt()], outs=[hfull.ap().opt()])

            def _once():
                # ============ Phase A: input MLP (h0 = LN(relu(x W + b))) ============
                for sw in range(n_sw):
                    w0 = sw * SW
                    nw = min(SW, WPC - w0)
                    nwf = nw * 128
                    ps_z = psum.tile([128, SW * 128], dt.float32, tag="ps_acc")
                    xt4 = stream.tile([128, SW * 128], dt.bfloat16, tag="m4", name="xt4")
                    nc.sync.dma_start(xt4[:, :nwf],
                                      xT_d.ap()[:, w0 * 128:w0 * 128 + nwf])
                    for j in range(nw):
                        nc.tensor.matmul(ps_z[:, j * 128:(j + 1) * 128],
                                         xt4[:, j * 128:(j + 1) * 128],
                                         w_in[:], start=True, stop=True)
                    # relu (+ optional b_in) -> r
                    r4 = stream.tile([128, SW * 128], dt.bfloat16, tag="u4", name="r4")
                    if "bin" in gb_t:
                        b3 = gb_t["bin"][:].unsqueeze(1).broadcast_to([128, nw, 128])
                        z3 = ps_z[:, :nwf].rearrange("p (w f) -> p w f", w=nw)
                        nc.vector.tensor_tensor(z3, z3, b3, AOp.add)
                    nc.scalar.activation(r4[:, :nwf], ps_z[:, :nwf], AF.Relu)
                    su = small.tile([128, SW], dt.float32, tag="su")
                    nc.vector.tensor_reduce(
                        su[:, :nw], r4[:, :nwf].rearrange("p (w f) -> p w f", w=nw),
                        mybir.AxisListType.X, AOp.add)
                    sqs = stream.tile([128, SW * 128], dt.bfloat16, tag="e4", name="sqs")
                    nc.scalar.activation(sqs[:, :nwf], r4[:, :nwf], AF.Square)
                    sq = small.tile([128, SW], dt.float32, tag="sq")
                    nc.vector.tensor_reduce(
                        sq[:, :nw], sqs[:, :nwf].rearrange("p (w f) -> p w f", w=nw),
                        mybir.AxisListType.X, AOp.add)
                    mean, rs = ln_smalls(su[:, :nw], sq[:, :nw], nw)
                    apply_ln(h_own[:, w0 * 128:w0 * 128 + nwf], r4[:, :nwf],
                             mean, rs, nw,
                             "gin" if "gin" in gb_t else None,
                             "bein" if "bein" in gb_t else None)
                    nc.sync.dma_start(
                        bounce0.ap()[w0 * 128:w0 * 128 + nwf, :]
                            .rearrange("(w p) f -> p w f", w=nw),
                        h_own[:, w0 * 128:w0 * 128 + nwf]
                            .rearrange("p (w f) -> p w f", w=nw))
                    ps_t4 = pst.tile([128, SW * 128], dt.bfloat16, tag="ps_t")
                    for j in range(nw):
                        w = w0 + j
                        nc.tensor.transpose(ps_t4[:, j * 128:(j + 1) * 128],
                                            h_own[:, w * 128:(w + 1) * 128], ident[:])
                    nc.scalar.activation(hT[:, w0 * 128:w0 * 128 + nwf],
                                         ps_t4[:, :nwf], AF.Copy)

                if debug_dumps:
                    nc.sync.dma_start(
                        dbg["dbg_h0"].ap().rearrange("(w p) f -> p w f", w=WPC),
                        h_own[:].rearrange("p (w f) -> p w f", w=WPC))
                _ag(bounce0, hfull0)
                if debug_dumps:
                    nc.sync.dma_start(dbg["dbg_hf0"].ap(), hfull0.ap())

                # ============ Phase B/C: conv layers ============
                for l in range(2):
                    hfull = (hfull0, hfull1)[l]
                    for sw in range(n_sw):
                        w0 = sw * SW
                        nw = min(SW, WPC - w0)
                        nwf = nw * 128
                        runs = sched[sw]
                        c0 = runs[0][1]
                        c1 = runs[-1][1] + sum(n for _, n in runs[-1][2])
                        TS = c1 - c0
                        # gather (one dma_gather per source chunk) + onehot
                        g_t = stream.tile([128, TS * 128], dt.bfloat16, tag="g")
                        idx_sw = tiny.tile([128, TS * 8], dt.int16, tag="idx_sw")
                        nc.sync.dma_start(idx_sw[:], idx_d.ap()[:, c0 * 8:c1 * 8])
                        if "gather" in ABLATE:
                            nc.sync.dma_start(
                                g_t[:].rearrange("p (t f) -> p t f", t=TS),
                                hfull.ap()[0:TS * 128, :]
                                    .rearrange("(t p) f -> p t f", p=128))
                        for gi, (c, t0, wruns) in enumerate(
                                [] if "gather" in ABLATE else runs):
                            nt = sum(n for _, n in wruns)
                            rows = min(CROWS, NFULL - c * CROWS)
                            nc.gpsimd.dma_gather(
                                g_t[:, (t0 - c0) * 128:(t0 - c0 + nt) * 128]
                                    .rearrange("p (t f) -> p t f", t=nt),
                                hfull.ap()[c * CROWS:c * CROWS + rows, :],
                                idx_sw[:, (t0 - c0) * 8:(t0 - c0 + nt) * 8],
                                nt * 128, nt * 128, F, single_packet=False,
                                queue_num=gi % 4)
                        oh_t = stream.tile([128, TS * 128], dt.bfloat16, tag="oh")
                        # pair views keep every last dim stride-1 so the DVE
                        # runs is_equal in 2x mode
                        oh4 = oh_t[:].rearrange("p (t f2 r) -> p t f2 r",
                                                t=TS, r=2)
                        iota_b = iota_t[:].rearrange("p (f2 r) -> p f2 r", r=2) \
                            .unsqueeze(1).broadcast_to([128, TS, 64, 2])
                        rel_b = rel_t[:, 2 * c0:2 * c1] \
                            .rearrange("p (t r) -> p t r", r=2) \
                            .unsqueeze(2).broadcast_to([128, TS, 64, 2])
                        if "onehot" not in ABLATE:
                            nc.vector.tensor_tensor(oh4, iota_b, rel_b, AOp.is_equal)
                        else:
                            nc.vector.memset(oh_t[:], 0.0)
                        if debug_dumps and l == 0:
                            nc.sync.dma_start(dbg["dbg_g0"].ap()[:, c0 * 128:c1 * 128],
                                              g_t[:])
                            nc.sync.dma_start(dbg["dbg_oh0"].ap()[:, c0 * 128:c1 * 128],
                                              oh_t[:])
                        # segment matmuls, window-major so each window's PSUM
                        # accumulation group opens and closes contiguously
                        ps_m = psum.tile([128, SW * 128], dt.float32, tag="ps_acc")
                        for j in range(nw):
                            w = w0 + j
                            wtiles = [t for t in range(c0, c1) if tile_win[t] == w]
                            if "seg" in ABLATE:
                                wtiles = wtiles[:1]
                            for i, t in enumerate(wtiles):
                                tl = t - c0
                                nc.tensor.matmul(
                                    ps_m[:, j * 128:(j + 1) * 128],
                                    oh_t[:, tl * 128:(tl + 1) * 128],
                                    g_t[:, tl * 128:(tl + 1) * 128],
                                    start=(i == 0), stop=(i == len(wtiles) - 1))
                        # m (scaled) node-major
                        m4 = stream.tile([128, SW * 128], dt.bfloat16, tag="m4")
                        m3 = m4[:, :nwf].rearrange("p (w f) -> p w f", w=nw)
                        iv = inv_t[:, w0:w0 + nw].unsqueeze(2).broadcast_to(
                            [128, nw, 128])
                        nc.vector.tensor_tensor(
                            m3, ps_m[:, :nwf].rearrange("p (w f) -> p w f", w=nw),
                            iv, AOp.mult)
                        if debug_dumps and l == 0:
                            nc.sync.dma_start(
                                dbg["dbg_m0"].ap()[w0 * 128:w0 * 128 + nwf, :]
                                    .rearrange("(w p) f -> p w f", w=nw),
                                m3)
                        # mT batched transpose + tm matmuls
                        ps_t4m = pst.tile([128, SW * 128], dt.bfloat16, tag="ps_t")
                        for j in range(nw):
                            nc.tensor.transpose(ps_t4m[:, j * 128:(j + 1) * 128],
                                                m4[:, j * 128:(j + 1) * 128],
                                                ident[:])
                        mt4 = tiny.tile([128, SW * 128], dt.bfloat16, tag="mt")
                        nc.scalar.activation(mt4[:, :nwf], ps_t4m[:, :nwf], AF.Copy)
                        ps_tm = psum.tile([128, SW * CH], dt.float32, tag="ps_sm")
                        for j in range(nw):
                            w = w0 + j
                            nc.tensor.matmul(ps_tm[:, j * CH:(j + 1) * CH],
                                             hT[:, w * 128:(w + 1) * 128],
                                             tmw[l][0][:], start=True, stop=False)
                            nc.tensor.matmul(ps_tm[:, j * CH:(j + 1) * CH],
                                             mt4[:, j * 128:(j + 1) * 128],
                                             tmw[l][1][:],
                                             start=False, stop=True)
                        nwc = nw * CH
                        if ("tmb0", "tmb1")[l] in gb_t:
                            tb = gb_t[("tmb0", "tmb1")[l]][:].unsqueeze(1) \
                                .broadcast_to([128, nw, CH])
                            z3 = ps_tm[:, :nwc].rearrange("p (w c) -> p w c", w=nw)
                            nc.vector.tensor_tensor(z3, z3, tb, AOp.add)
                        # softmax (no max-sub) + cumsum
                        e4 = stream.tile([128, SW * CH], dt.float32, tag="e4")
                        nc.scalar.activation(e4[:, :nwc], ps_tm[:, :nwc], AF.Exp)
                        s4 = small.tile([128, SW], dt.float32, tag="s4")
                        nc.vector.tensor_reduce(
                            s4[:, :nw], e4[:, :nwc].rearrange("p (w c) -> p w c", w=nw),
                            mybir.AxisListType.X, AOp.add)
                        r4s = small.tile([128, SW], dt.float32, tag="r4s")
                        nc.vector.reciprocal(r4s[:, :nw], s4[:, :nw])
                        cs4 = stream.tile([128, SW * CH], dt.float32, tag="cs4")
                        for j in range(nw):
                            nc.vector.tensor_tensor_scan(
                                cs4[:, j * CH:(j + 1) * CH],
                                e4[:, j * CH:(j + 1) * CH],
                                e4[:, j * CH:(j + 1) * CH], 0.0, AOp.add, AOp.bypass)
                        # sig update
                        rb = r4s[:, :nw].unsqueeze(2).broadcast_to([128, nw, CH])
                        cs3 = cs4[:, :nwc].rearrange("p (w c) -> p w c", w=nw)
                        sg_cols = sigc[:, w0 * CH:w0 * CH + nwc]
                        sg3 = sg_cols.rearrange("p (w c) -> p w c", w=nw)
                        if l == 0:
                            nc.vector.tensor_tensor(sg3, cs3, rb, AOp.mult)
                            sig_src = sg_cols
                        else:
                            t4 = stream.tile([128, SW * CH], dt.bfloat16, tag="t4")
                            t3 = t4[:, :nwc].rearrange("p (w c) -> p w c", w=nw)
                            nc.vector.tensor_tensor(t3, cs3, rb, AOp.mult)
                            a4 = stream.tile([128, SW * CH], dt.bfloat16, tag="a4")
                            nc.vector.tensor_tensor(a4[:, :nwc], sg_cols, t4[:, :nwc],
                                                    AOp.mult)
                            nc.vector.tensor_tensor(t4[:, :nwc], t4[:, :nwc],
                                                    a4[:, :nwc], AOp.subtract)
                            nc.vector.tensor_tensor(t4[:, :nwc], t4[:, :nwc],
                                                    sg_cols, AOp.add)
                            sig_src = t4[:, :nwc]
                        # mix u = h*sig + m*(1-sig) = m + sig*(h-m)
                        # sig broadcast over the repeat axis (64 -> 128)
                        sig_b = sig_src.rearrange("p (w c) -> p w c", w=nw) \
                            .unsqueeze(3).broadcast_to([128, nw, CH, 2])
                        hcols = h_own[:, w0 * 128:w0 * 128 + nwf]
                        u4 = stream.tile([128, SW * 128], dt.bfloat16, tag="u4")
                        nc.vector.tensor_tensor(u4[:, :nwf], hcols, m4[:, :nwf],
                                                AOp.subtract)
                        u4d = u4[:, :nwf].rearrange("p (w c r) -> p w c r",
                                                    w=nw, r=2)
                        nc.vector.tensor_tensor(u4d, u4d, sig_b, AOp.mult)
                        nc.vector.tensor_tensor(u4[:, :nwf], u4[:, :nwf], m4[:, :nwf],
                                                AOp.add)
                        # LN stats
                        su = small.tile([128, SW], dt.float32, tag="su")
                        nc.vector.tensor_reduce(
                            su[:, :nw], u4[:, :nwf].rearrange("p (w f) -> p w f", w=nw),
                            mybir.AxisListType.X, AOp.add)
                        sqs = stream.tile([128, SW * 128], dt.bfloat16, tag="e4", name="sqs")
                        nc.scalar.activation(sqs[:, :nwf], u4[:, :nwf], AF.Square)
                        sq = small.tile([128, SW], dt.float32, tag="sq")
                        nc.vector.tensor_reduce(
                            sq[:, :nw], sqs[:, :nwf].rearrange("p (w f) -> p w f", w=nw),
                            mybir.AxisListType.X, AOp.add)
                        mean, rs = ln_smalls(su[:, :nw], sq[:, :nw], nw)
                        gnm = ("lng0", "lng1")[l]
                        bnm = ("lnb0", "lnb1")[l]
                        if l == 0:
                            apply_ln(hcols, u4[:, :nwf], mean, rs, nw,
                                     gnm if gnm in gb_t else None,
                                     bnm if bnm in gb_t else None)
                            nc.sync.dma_start(
                                bounce1.ap()[w0 * 128:w0 * 128 + nwf, :]
                                    .rearrange("(w p) f -> p w f", w=nw),
                                hcols.rearrange("p (w f) -> p w f", w=nw))
                            ps_t4 = pst.tile([128, SW * 128], dt.bfloat16, tag="ps_t")
                            for j in range(nw):
                                w = w0 + j
                                nc.tensor.transpose(
                                    ps_t4[:, j * 128:(j + 1) * 128],
                                    h_own[:, w * 128:(w + 1) * 128], ident[:])
                            nc.scalar.activation(hT[:, w0 * 128:w0 * 128 + nwf],
                                                 ps_t4[:, :nwf], AF.Copy)
                        else:
                            h2 = stream.tile([128, SW * 128], dt.bfloat16, tag="hx", name="h2")
                            apply_ln(h2[:, :nwf], u4[:, :nwf], mean, rs, nw,
                                     gnm if gnm in gb_t else None,
                                     bnm if bnm in gb_t else None)
                            ob = stream.tile([128, SW * OUT_C], dt.float32, tag="ob")
                            ps_o = psum.tile([128, SW * OUT_C], dt.float32, tag="ps_sm")
                            ps_t4 = pst.tile([128, SW * 128], dt.bfloat16, tag="ps_t")
                            for j in range(nw):
                                nc.tensor.transpose(
                                    ps_t4[:, j * 128:(j + 1) * 128],
                                    h2[:, j * 128:(j + 1) * 128], ident[:])
                            h2t4 = tiny.tile([128, SW * 128], dt.bfloat16, tag="h2t")
                            nc.scalar.activation(h2t4[:, :nwf], ps_t4[:, :nwf],
                                                 AF.Copy)
                            for j in range(nw):
                                nc.tensor.matmul(ps_o[:, j * OUT_C:(j + 1) * OUT_C],
                                                 h2t4[:, j * 128:(j + 1) * 128],
                                                 wout[:], start=True, stop=True)
                            nwo = nw * OUT_C
                            if "bout" in gb_t:
                                bb = gb_t["bout"][:].unsqueeze(1).broadcast_to(
                                    [128, nw, OUT_C])
                                o3 = ob[:, :nwo].rearrange("p (w o) -> p w o", w=nw)
                                nc.vector.tensor_tensor(
                                    o3, ps_o[:, :nwo].rearrange("p (w o) -> p w o", w=nw),
                                    bb, AOp.add)
                            else:
                                nc.vector.tensor_copy(ob[:, :nwo], ps_o[:, :nwo])
                            nc.sync.dma_start(
                                out_d.ap()[w0 * 128:w0 * 128 + nwf, :]
                                    .rearrange("(w p) o -> p w o", w=nw),
                                ob[:, :nwo].rearrange("p (w o) -> p w o", w=nw))
                    if l == 0:
                        _ag(bounce1, hfull1)
                    if l == 0 and debug_dumps:
                        nc.sync.dma_start(dbg["dbg_sig0"].ap(), sigc[:])
                        nc.sync.dma_start(
                            dbg["dbg_h1"].ap().rearrange("(w p) f -> p w f", w=WPC),
                            h_own[:].rearrange("p (w f) -> p w f", w=WPC))


            for _rep in range(repeat):
                _once()

    nc.compile()
    return nc


_CACHE = {}


def _sched_key(prep):
    return (tuple(prep["tile_win"]),
            tuple((c, t0, tuple(wr)) for sw in prep["sched"]
                  for c, t0, wr in sw))


def _get_compiled(cfg, prep, flags):
    key = (_sched_key(prep), tuple(sorted(flags.items())))
    if key not in _CACHE:
        _CACHE[key] = build_nc(cfg, prep, flags)
    return _CACHE[key]


class PjrtRunner:
    """Persistent jitted shard_map executor for one compiled nc (8 cores)."""

    def __init__(self, nc, donate=True):
        import jax
        from jax.experimental.shard_map import shard_map
        from jax.sharding import Mesh, PartitionSpec
        from concourse import bass2jax

        bass2jax.install_neuronx_cc_hook()
        self.nc = nc
        in_names, out_names, out_avals, zero_outs = [], [], [], []
        partition_name = (nc.partition_id_tensor.name
                          if nc.partition_id_tensor else None)
        for alloc in nc.m.functions[0].allocations:
            if not isinstance(alloc, mybir.MemoryLocationSet):
                continue
            name = alloc.memorylocations[0].name
            if alloc.kind == "ExternalInput":
                if name != partition_name:
                    in_names.append(name)
            elif alloc.kind == "ExternalOutput":
                import jax.core as jcore
                out_names.append(name)
                aval = jax.core.ShapedArray(
                    tuple(alloc.tensor_shape), mybir.dt.np(alloc.dtype))
                out_avals.append(aval)
                zero_outs.append(np.zeros(alloc.tensor_shape,
                                          mybir.dt.np(alloc.dtype)))
        self.n_params = len(in_names)
        self.out_names = list(out_names)
        self.zero_outs = zero_outs
        all_in = in_names + out_names
        if partition_name is not None:
            all_in.append(partition_name)
        self.in_names_data = in_names

        def _body(*args):
            operands = list(args)
            if partition_name is not None:
                operands.append(bass2jax.partition_id_tensor())
            outs = bass2jax._bass_exec_p.bind(
                *operands,
                out_avals=tuple(out_avals),
                in_names=tuple(all_in),
                out_names=tuple(out_names),
                lowering_input_output_aliases=(),
                sim_require_finite=True,
                sim_require_nnan=True,
                nc=nc,
            )
            return tuple(outs)

        devices = jax.devices()[:NCORES]
        self.mesh = Mesh(np.asarray(devices), ("core",))
        n_out = len(out_names)
        donate_nums = (tuple(range(self.n_params, self.n_params + n_out))
                       if donate else ())
        in_specs = (PartitionSpec("core"),) * (self.n_params + n_out)
        out_specs = (PartitionSpec("core"),) * n_out
        self.fn = jax.jit(
            shard_map(_body, mesh=self.mesh, in_specs=in_specs,
                      out_specs=out_specs, check_rep=False),
            donate_argnums=donate_nums, keep_unused=True)

    def concat_inputs(self, in_maps):
        return [
            np.concatenate([np.asarray(in_maps[c][nm]) for c in range(NCORES)],
                           axis=0)
            for nm in self.in_names_data
        ]

    def zeros(self):
        return [np.zeros((NCORES * z.shape[0], *z.shape[1:]), z.dtype)
                for z in self.zero_outs]

    def __call__(self, concat_in, zeros):
        import jax
        outs = self.fn(*concat_in, *zeros)
        return {nm: np.asarray(outs[i]) for i, nm in enumerate(self.out_names)}


_RUNNERS = {}


def get_runner(cfg, prep, flags):
    key = (_sched_key(prep), tuple(sorted(flags.items())))
    if key not in _RUNNERS:
        _RUNNERS[key] = PjrtRunner(_get_compiled(cfg, prep, flags))
    return _RUNNERS[key]


def run(inputs, cfg):
    x = np.asarray(inputs["x"], np.float32)
    prep = _host_prep(x, np.asarray(inputs["edge_index"]), cfg)
    SH = cfg["SH"]

    flags = make_flags(inputs)
    runner = get_runner(cfg, prep, flags)
    in_maps = make_in_maps(inputs, prep, flags)
    out = runner(runner.concat_inputs(in_maps), runner.zeros())["out"]
    SHP = prep["SHP"]
    out = out.reshape(NCORES, SHP, OUT_C)[:, :SH, :]
    return np.ascontiguousarray(out.reshape(NCORES * SH, OUT_C), dtype=np.float32)


def make_flags(inputs):
    return {
        "bin_triv": _affine_trivial(1.0, inputs["b_in"]),
        "gin_triv": _affine_trivial(inputs["g_in"], 0.0),
        "bein_triv": _affine_trivial(1.0, inputs["be_in"]),
        "lng0_triv": _affine_trivial(inputs["ln_g0"], 0.0),
        "lnb0_triv": _affine_trivial(1.0, inputs["ln_b0"]),
        "lng1_triv": _affine_trivial(inputs["ln_g1"], 0.0),
        "lnb1_triv": _affine_trivial(1.0, inputs["ln_b1"]),
        "tmb0_triv": _affine_trivial(1.0, inputs["tm_b0"]),
        "tmb1_triv": _affine_trivial(1.0, inputs["tm_b1"]),
        "bout_triv": _affine_trivial(1.0, inputs["b_out"]),
    }


def make_in_maps(inputs, prep, flags):
    def bc(v, width):
        return np.tile(np.asarray(v, np.float32).reshape(1, width), (128, 1))

    in_maps = []
    for k in range(NCORES):
        import ml_dtypes
        bf16 = ml_dtypes.bfloat16
        m = {
            "xT": prep["xT"][k].astype(bf16),
            "w_in": np.asarray(inputs["W_in"], np.float32).astype(bf16),
            "idx16": prep["idx16"][k],
            "rel": prep["rel"][k].astype(bf16),
            "inv": prep["inv"][k],
            "tmw0": np.asarray(inputs["tm_W0"], np.float32).astype(bf16),
            "tmw1": np.asarray(inputs["tm_W1"], np.float32).astype(bf16),
            "wout": np.asarray(inputs["W_out"], np.float32).astype(bf16),
            "iota": np.tile(np.arange(128, dtype=np.float32)[None, :],
                            (128, 1)).astype(bf16),
            "ident": np.eye(128, dtype=np.float32).astype(bf16),
        }
        if not flags["bin_triv"]:
            m["bin"] = bc(inputs["b_in"], F)
        if not flags["gin_triv"]:
            m["gin"] = bc(inputs["g_in"], F)
        if not flags["bein_triv"]:
            m["bein"] = bc(inputs["be_in"], F)
        for nm, src in (("lng0", "ln_g0"), ("lnb0", "ln_b0"),
                        ("lng1", "ln_g1"), ("lnb1", "ln_b1")):
            if not flags[nm + "_triv"]:
                m[nm] = bc(inputs[src], F)
        if not flags["tmb0_triv"]:
            m["tmb0"] = bc(inputs["tm_b0"], CH)
        if not flags["tmb1_triv"]:
            m["tmb1"] = bc(inputs["tm_b1"], CH)
        if not flags["bout_triv"]:
            m["bout"] = bc(inputs["b_out"], OUT_C)
        in_maps.append(m)
    return in_maps


def kernel(**inputs):
    return run(inputs, FULL_CFG)



# revision 24
# speedup vs baseline: 1.7515x; 1.1151x over previous
"""ONGNN (2-layer ordered-neuron GNN) on 8 Trainium2 NeuronCores.

Strategy: shard DESTINATION nodes across the 8 cores (12500/core, padded to
12544 = 98*128).  Edges are bucketed on the host by (core, dst-window-of-128);
per-window message-tile counts are equalized across cores so one SPMD program
serves all cores.  Each conv layer:
  - AllGather of the bf16 node-feature shards -> full table in each core's DRAM
  - indirect-DMA gather of source rows (one 256B row per edge)
  - segment-sum via one-hot matmuls accumulated in PSUM (one-hot built on-chip
    from dst indices with a broadcast is_equal)
  - node-parallel dense math (transition matmul, softmax, cumsum, gating mix,
    layernorm) batched over superwindows of 4x128 nodes.
"""
import sys
import numpy as np

sys.path.insert(0, "/opt/trn_rl_repo")

import concourse.bass as bass
import concourse.bacc as bacc
import concourse.mybir as mybir
import concourse.tile as tile
from concourse import bass_utils

F = 128       # feature dim (IN_C == HID)
CH = 64       # CHUNK
OUT_C = 40
EPS = 1e-5
NCORES = 8

FULL_CFG = dict(N=100000, E=1000000, SH=12500, WPC=98, SW=4)
# SH: dst nodes per core; WPC: 128-node windows per core (ceil(SH/128));
# SW: windows per superwindow (batching factor for elementwise ops).

ABLATE = set()
STREAM_BUFS = 2
TINY_BUFS = 4
AOp = None  # filled lazily
AF = None


def _host_prep(x, edge_index, cfg):
    """Bucket edges by (core, window, src-chunk), build device arrays and the
    shared tile schedule.  Message stream order per superwindow: for each
    source-table chunk, for each window in the superwindow, that (w,c) run's
    tiles (padded to 128).  One dma_gather call covers one (sw, chunk) run."""
    N, E, SH, WPC, SW = cfg["N"], cfg["E"], cfg["SH"], cfg["WPC"], cfg["SW"]
    SHP = WPC * 128
    NFULL = NCORES * SHP
    CROWS = max(SHP, (32767 // SHP) * SHP)      # chunk rows (int16-addressable)
    NCH = -(-NFULL // CROWS)
    n_sw = -(-WPC // SW)
    src = np.asarray(edge_index[0], dtype=np.int64)
    dst = np.asarray(edge_index[1], dtype=np.int64)

    core = dst // SH
    dst_loc = dst - core * SH
    win = dst_loc >> 7
    grow = (src // SH) * SHP + (src % SH)       # padded full-table row
    chunk = grow // CROWS
    bucket = ((core * WPC + win) * NCH + chunk).astype(np.int64)
    order = np.argsort(bucket, kind="stable")
    bcnt = np.bincount(bucket, minlength=NCORES * WPC * NCH) \
        .reshape(NCORES, WPC, NCH)
    tpwc = -(-bcnt // 128)
    tpwc = tpwc.max(axis=0)                      # [WPC, NCH]
    for w in range(WPC):
        if tpwc[w].sum() == 0:
            tpwc[w, 0] = 1

    # schedule: per sw, per chunk, the window runs; global tile offsets
    sched = []
    t_acc = 0
    tile_win = []                                # window id of every tile
    for sw in range(n_sw):
        w0, w1 = sw * SW, min((sw + 1) * SW, WPC)
        runs = []
        for c in range(NCH):
            wruns = [(w, int(tpwc[w, c])) for w in range(w0, w1)
                     if tpwc[w, c] > 0]
            nt = sum(n for _, n in wruns)
            if nt == 0:
                continue
            runs.append((c, t_acc, wruns))
            for w, n in wruns:
                tile_win.extend([w] * n)
            t_acc += nt
        sched.append(runs)
    T = t_acc

    starts = np.zeros(NCORES * WPC * NCH + 1, np.int64)
    np.cumsum(bcnt.reshape(-1), out=starts[1:])
    idx16 = np.zeros((NCORES, T * 128), np.int16)
    rel = np.full((NCORES, T * 128), 512.0, np.float32)
    for k in range(NCORES):
        for sw in range(n_sw):
            for c, t0, wruns in sched[sw]:
                pos = t0 * 128
                for w, ntile in wruns:
                    b = (k * WPC + w) * NCH + c
                    sel = order[starts[b]:starts[b + 1]]
                    sel = sel[np.argsort(grow[sel], kind="stable")]
                    n = sel.size
                    idx16[k, pos:pos + n] = (grow[sel] - c * CROWS) \
                        .astype(np.int16)
                    rel[k, pos:pos + n] = (dst_loc[sel] - w * 128) \
                        .astype(np.float32)
                    pos += ntile * 128
    # device layouts
    idx16_dev = np.ascontiguousarray(idx16.reshape(NCORES, T * 8, 16)
                                     .transpose(0, 2, 1))     # [NC,16,T*8]
    idx16_dev = np.tile(idx16_dev, (1, 8, 1))                 # [NC,128,T*8]
    rel_dev = np.ascontiguousarray(rel.reshape(NCORES, T, 128)
                                   .transpose(0, 2, 1))       # [NC,128,T]
    # each rel value stored twice so the on-chip is_equal can use a
    # contiguous pair view (DVE 2x mode needs last-dim stride 1)
    rel_dev = np.repeat(rel_dev, 2, axis=2)                   # [NC,128,2T]

    cnt = np.bincount(dst, minlength=N).astype(np.float32)
    inv = 1.0 / np.maximum(cnt, 1.0)
    inv_dev = np.ones((NCORES, SHP), np.float32)
    for k in range(NCORES):
        inv_dev[k, :SH] = inv[k * SH:(k + 1) * SH]
    inv_dev = inv_dev.reshape(NCORES, WPC, 128).transpose(0, 2, 1)
    inv_dev = np.ascontiguousarray(inv_dev)

    xT_dev = np.zeros((NCORES, F, SHP), np.float32)
    x = np.asarray(x, np.float32)
    for k in range(NCORES):
        xT_dev[k, :, :SH] = x[k * SH:(k + 1) * SH].T

    return dict(idx16=idx16_dev, rel=rel_dev, inv=inv_dev, xT=xT_dev,
                sched=sched, tile_win=tile_win, T=T, SHP=SHP, CROWS=CROWS,
                tpwc=tpwc)


def _affine_trivial(g, b):
    return bool(np.allclose(g, 1.0, atol=1e-7) and np.allclose(b, 0.0, atol=1e-7))


def build_nc(cfg, prep, flags, debug_dumps=False, no_collectives=False, repeat=1):
    """Build the SPMD Bass program. flags: dict of *_trivial booleans."""
    global AOp, AF
    AOp = mybir.AluOpType
    AF = mybir.ActivationFunctionType
    dt = mybir.dt

    WPC, SW = cfg["WPC"], cfg["SW"]
    SHP = WPC * 128
    T = prep["T"]
    sched, tile_win, CROWS = prep["sched"], prep["tile_win"], prep["CROWS"]
    NFULL = NCORES * SHP

    nc = bacc.Bacc("TRN2", target_bir_lowering=False, debug=False,
                   num_devices=NCORES, num_swdge_queues=4)

    # ---- I/O ----
    xT_d = nc.dram_tensor("xT", [F, SHP], dt.bfloat16, kind="ExternalInput")
    win_d = nc.dram_tensor("w_in", [F, F], dt.bfloat16, kind="ExternalInput")
    idx_d = nc.dram_tensor("idx16", [128, T * 8], dt.int16, kind="ExternalInput")
    rel_d = nc.dram_tensor("rel", [128, 2 * T], dt.bfloat16, kind="ExternalInput")
    inv_d = nc.dram_tensor("inv", [128, WPC], dt.float32, kind="ExternalInput")
    tmw0_d = nc.dram_tensor("tmw0", [2 * F, CH], dt.bfloat16, kind="ExternalInput")
    tmw1_d = nc.dram_tensor("tmw1", [2 * F, CH], dt.bfloat16, kind="ExternalInput")
    wout_d = nc.dram_tensor("wout", [F, OUT_C], dt.bfloat16, kind="ExternalInput")
    iota_d = nc.dram_tensor("iota", [128, 128], dt.bfloat16, kind="ExternalInput")
    ident_d = nc.dram_tensor("ident", [128, 128], dt.bfloat16, kind="ExternalInput")
    out_d = nc.dram_tensor("out", [SHP, OUT_C], dt.float32, kind="ExternalOutput")
    gb_d = {}
    for nm in ("bin", "gin", "bein", "lng0", "lnb0", "lng1", "lnb1", "tmb0",
               "tmb1", "bout"):
        if not flags[nm + "_triv"]:
            width = {"tmb0": CH, "tmb1": CH, "bout": OUT_C}.get(nm, F)
            gb_d[nm] = nc.dram_tensor(nm, [128, width], dt.float32,
                                      kind="ExternalInput")

    dbg = {}
    if debug_dumps:
        for nm, shape, d in (("dbg_h0", [SHP, F], dt.float32),
                             ("dbg_hf0", [NFULL, F], dt.bfloat16),
                             ("dbg_m0", [SHP, F], dt.float32),
                             ("dbg_sig0", [128, WPC * CH], dt.float32),
                             ("dbg_g0", [128, 0], dt.bfloat16),
                             ("dbg_oh0", [128, 0], dt.bfloat16),
                             ("dbg_h1", [SHP, F], dt.float32)):
            if nm in ("dbg_g0", "dbg_oh0"):
                shape = [128, T * 128]
            dbg[nm] = nc.dram_tensor(nm, shape, d, kind="ExternalOutput")
    bounce0 = nc.dram_tensor("bounce0", [SHP, F], dt.bfloat16)
    bounce1 = nc.dram_tensor("bounce1", [SHP, F], dt.bfloat16)
    hfull0 = nc.dram_tensor("hfull0", [NFULL, F], dt.bfloat16, addr_space="Shared")
    hfull1 = nc.dram_tensor("hfull1", [NFULL, F], dt.bfloat16, addr_space="Shared")

    n_sw = (WPC + SW - 1) // SW

    with tile.TileContext(nc) as tc:
        import contextlib
        ctx = contextlib.ExitStack()
        with ctx:
            ctx.enter_context(nc.allow_low_precision(
                reason="bf16 elementwise; LN stats accumulate in fp32"))
            res = ctx.enter_context(tc.tile_pool(name="res", bufs=1))
            stream = ctx.enter_context(tc.tile_pool(name="stream", bufs=STREAM_BUFS))
            tiny = ctx.enter_context(tc.tile_pool(name="tiny", bufs=TINY_BUFS))
            small = ctx.enter_context(tc.tile_pool(name="small", bufs=3))
            psum = ctx.enter_context(tc.tile_pool(name="psum", bufs=2, space="PSUM"))
            pst = ctx.enter_context(tc.tile_pool(name="pst", bufs=2, space="PSUM"))

            # ---- residents / constants ----
            hT = res.tile([128, SHP], dt.bfloat16, tag="hT")       # feat-major own shard
            h_own = res.tile([128, SHP], dt.bfloat16, tag="h_own") # node-major own shard
            sigc = res.tile([128, WPC * CH], dt.bfloat16, tag="sigc")
            rel_t = res.tile([128, 2 * T], dt.bfloat16, tag="rel")
            inv_t = res.tile([128, WPC], dt.float32, tag="inv")
            w_in = res.tile([F, F], dt.bfloat16, tag="w_in")
            tmw = [[res.tile([F, CH], dt.bfloat16, tag=f"tmw{l}{h}", name=f"tmw{l}{h}")
                    for h in range(2)] for l in range(2)]
            wout = res.tile([F, OUT_C], dt.bfloat16, tag="wout")
            iota_t = res.tile([128, 128], dt.bfloat16, tag="iota")
            ident = res.tile([128, 128], dt.bfloat16, tag="ident")
            gb_t = {}
            for nm, d in gb_d.items():
                gb_t[nm] = res.tile(list(d.shape), dt.float32, tag=nm, name=nm)
                nc.sync.dma_start(gb_t[nm][:], d.ap())

            nc.sync.dma_start(rel_t[:], rel_d.ap())
            nc.sync.dma_start(inv_t[:], inv_d.ap())
            nc.sync.dma_start(w_in[:], win_d.ap())
            for l, d in enumerate((tmw0_d, tmw1_d)):
                nc.sync.dma_start(tmw[l][0][:], d.ap()[0:F, :])
                nc.sync.dma_start(tmw[l][1][:], d.ap()[F:2 * F, :])
            nc.sync.dma_start(wout[:], wout_d.ap())
            nc.sync.dma_start(iota_t[:], iota_d.ap())
            nc.sync.dma_start(ident[:], ident_d.ap())
            eps_t = res.tile([128, 1], dt.float32, tag="eps")
            nc.vector.memset(eps_t[:], EPS)

            def ln_smalls(su, sq, nw):
                """su,sq: [128,nw] fp32 -> (mean, rs) each [128,nw] fp32"""
                mean = small.tile([128, nw], dt.float32, tag="mean")
                nc.vector.tensor_scalar(mean[:], su, 1.0 / F, None, AOp.mult)
                t1 = small.tile([128, nw], dt.float32, tag="t1")
                nc.vector.tensor_tensor(t1[:], mean[:], su, AOp.mult)
                t2 = small.tile([128, nw], dt.float32, tag="t2")
                nc.vector.tensor_tensor(t2[:], sq, t1[:], AOp.subtract)
                srt = small.tile([128, nw], dt.float32, tag="srt")
                nc.scalar.activation(srt[:], t2[:], AF.Sqrt, bias=eps_t[:],
                                     scale=1.0 / F)
                rs = small.tile([128, nw], dt.float32, tag="rs")
                nc.vector.reciprocal(rs[:], srt[:])
                return mean, rs

            def apply_ln(dst_ap, u_ap, mean, rs, nw, gnm, bnm):
                """dst = (u - mean)*rs [* g] [+ b]; u_ap/dst [128, nw*128] bf16.
                Per-window fused tensor_scalar keeps operands contiguous
                (2x DVE mode); scalars are per-partition APs."""
                if gnm is None and bnm is None:
                    for j in range(nw):
                        nc.vector.tensor_scalar(
                            dst_ap[:, j * 128:(j + 1) * 128],
                            u_ap[:, j * 128:(j + 1) * 128],
                            mean[:, j:j + 1], rs[:, j:j + 1],
                            AOp.subtract, AOp.mult)
                    return
                u3 = u_ap.rearrange("p (w f) -> p w f", w=nw)
                d3 = dst_ap.rearrange("p (w f) -> p w f", w=nw)
                mb = mean[:].unsqueeze(2).broadcast_to([128, nw, 128])
                rb = rs[:].unsqueeze(2).broadcast_to([128, nw, 128])
                nc.vector.tensor_tensor(d3, u3, mb, AOp.subtract)
                nc.vector.tensor_tensor(d3, d3, rb, AOp.mult)
                if gnm is not None:
                    g3 = gb_t[gnm][:].unsqueeze(1).broadcast_to([128, nw, 128])
                    nc.vector.tensor_tensor(d3, d3, g3, AOp.mult)
                if bnm is not None:
                    b3 = gb_t[bnm][:].unsqueeze(1).broadcast_to([128, nw, 128])
                    nc.vector.tensor_tensor(d3, d3, b3, AOp.add)

            def _ag(bounce, hfull):
                if no_collectives:
                    nc.sync.dma_start(hfull.ap()[0:SHP, :], bounce.ap())
                else:
                    nc.gpsimd.collective_compute(
                        "AllGather", AOp.bypass,
                        replica_groups=[list(range(NCORES))],
                        ins=[bounce.ap().opt()], outs=[hfull.ap().opt()])

            def _once():
                # ============ Phase A: input MLP (h0 = LN(relu(x W + b))) ============
                for sw in range(n_sw):
                    w0 = sw * SW
                    nw = min(SW, WPC - w0)
                    nwf = nw * 128
                    ps_z = psum.tile([128, SW * 128], dt.float32, tag="ps_acc")
                    xt4 = stream.tile([128, SW * 128], dt.bfloat16, tag="m4", name="xt4")
                    nc.sync.dma_start(xt4[:, :nwf],
                                      xT_d.ap()[:, w0 * 128:w0 * 128 + nwf])
                    for j in range(nw):
                        nc.tensor.matmul(ps_z[:, j * 128:(j + 1) * 128],
                                         xt4[:, j * 128:(j + 1) * 128],
                                         w_in[:], start=True, stop=True)
                    # relu (+ optional b_in) -> r
                    r4 = stream.tile([128, SW * 128], dt.bfloat16, tag="u4", name="r4")
                    if "bin" in gb_t:
                        b3 = gb_t["bin"][:].unsqueeze(1).broadcast_to([128, nw, 128])
                        z3 = ps_z[:, :nwf].rearrange("p (w f) -> p w f", w=nw)
                        nc.vector.tensor_tensor(z3, z3, b3, AOp.add)
                    nc.scalar.activation(r4[:, :nwf], ps_z[:, :nwf], AF.Relu)
                    su = small.tile([128, SW], dt.float32, tag="su")
                    nc.vector.tensor_reduce(
                        su[:, :nw], r4[:, :nwf].rearrange("p (w f) -> p w f", w=nw),
                        mybir.AxisListType.X, AOp.add)
                    sqs = stream.tile([128, SW * 128], dt.bfloat16, tag="e4", name="sqs")
                    nc.scalar.activation(sqs[:, :nwf], r4[:, :nwf], AF.Square)
                    sq = small.tile([128, SW], dt.float32, tag="sq")
                    nc.vector.tensor_reduce(
                        sq[:, :nw], sqs[:, :nwf].rearrange("p (w f) -> p w f", w=nw),
                        mybir.AxisListType.X, AOp.add)
                    mean, rs = ln_smalls(su[:, :nw], sq[:, :nw], nw)
                    apply_ln(h_own[:, w0 * 128:w0 * 128 + nwf], r4[:, :nwf],
                             mean, rs, nw,
                             "gin" if "gin" in gb_t else None,
                             "bein" if "bein" in gb_t else None)
                    nc.sync.dma_start(
                        bounce0.ap()[w0 * 128:w0 * 128 + nwf, :]
                            .rearrange("(w p) f -> p w f", w=nw),
                        h_own[:, w0 * 128:w0 * 128 + nwf]
                            .rearrange("p (w f) -> p w f", w=nw))
                    ps_t4 = pst.tile([128, SW * 128], dt.bfloat16, tag="ps_t")
                    for j in range(nw):
                        w = w0 + j
                        nc.tensor.transpose(ps_t4[:, j * 128:(j + 1) * 128],
                                            h_own[:, w * 128:(w + 1) * 128], ident[:])
                    nc.scalar.activation(hT[:, w0 * 128:w0 * 128 + nwf],
                                         ps_t4[:, :nwf], AF.Copy)

                if debug_dumps:
                    nc.sync.dma_start(
                        dbg["dbg_h0"].ap().rearrange("(w p) f -> p w f", w=WPC),
                        h_own[:].rearrange("p (w f) -> p w f", w=WPC))
                _ag(bounce0, hfull0)
                if debug_dumps:
                    nc.sync.dma_start(dbg["dbg_hf0"].ap(), hfull0.ap())

                # ============ Phase B/C: conv layers ============
                for l in range(2):
                    hfull = (hfull0, hfull1)[l]
                    for sw in range(n_sw):
                        w0 = sw * SW
                        nw = min(SW, WPC - w0)
                        nwf = nw * 128
                        runs = sched[sw]
                        c0 = runs[0][1]
                        c1 = runs[-1][1] + sum(n for _, n in runs[-1][2])
                        TS = c1 - c0
                        # gather (one dma_gather per source chunk) + onehot
                        g_t = stream.tile([128, TS * 128], dt.bfloat16, tag="g")
                        idx_sw = tiny.tile([128, TS * 8], dt.int16, tag="idx_sw")
                        nc.sync.dma_start(idx_sw[:], idx_d.ap()[:, c0 * 8:c1 * 8])
                        if "gather" in ABLATE:
                            nc.sync.dma_start(
                                g_t[:].rearrange("p (t f) -> p t f", t=TS),
                                hfull.ap()[0:TS * 128, :]
                                    .rearrange("(t p) f -> p t f", p=128))
                        for gi, (c, t0, wruns) in enumerate(
                                [] if "gather" in ABLATE else runs):
                            nt = sum(n for _, n in wruns)
                            rows = min(CROWS, NFULL - c * CROWS)
                            nc.gpsimd.dma_gather(
                                g_t[:, (t0 - c0) * 128:(t0 - c0 + nt) * 128]
                                    .rearrange("p (t f) -> p t f", t=nt),
                                hfull.ap()[c * CROWS:c * CROWS + rows, :],
                                idx_sw[:, (t0 - c0) * 8:(t0 - c0 + nt) * 8],
                                nt * 128, nt * 128, F, single_packet=False,
                                queue_num=gi % 4)
                        oh_t = stream.tile([128, TS * 128], dt.bfloat16, tag="oh")
                        # pair views keep every last dim stride-1 so the DVE
                        # runs is_equal in 2x mode
                        oh4 = oh_t[:].rearrange("p (t f2 r) -> p t f2 r",
                                                t=TS, r=2)
                        iota_b = iota_t[:].rearrange("p (f2 r) -> p f2 r", r=2) \
                            .unsqueeze(1).broadcast_to([128, TS, 64, 2])
                        rel_b = rel_t[:, 2 * c0:2 * c1] \
                            .rearrange("p (t r) -> p t r", r=2) \
                            .unsqueeze(2).broadcast_to([128, TS, 64, 2])
                        if "onehot" not in ABLATE:
                            nc.vector.tensor_tensor(oh4, iota_b, rel_b, AOp.is_equal)
                        else:
                            nc.vector.memset(oh_t[:], 0.0)
                        # segment matmuls, window-major so each window's PSUM
                        # accumulation group opens and closes contiguously
                        ps_m = psum.tile([128, SW * 128], dt.float32, tag="ps_acc")
                        for j in range(nw):
                            w = w0 + j
                            wtiles = [t for t in range(c0, c1) if tile_win[t] == w]
                            if "seg" in ABLATE:
                                wtiles = wtiles[:1]
                            for i, t in enumerate(wtiles):
                                tl = t - c0
                                nc.tensor.matmul(
                                    ps_m[:, j * 128:(j + 1) * 128],
                                    oh_t[:, tl * 128:(tl + 1) * 128],
                                    g_t[:, tl * 128:(tl + 1) * 128],
                                    start=(i == 0), stop=(i == len(wtiles) - 1))
                        # m (scaled) node-major
                        m4 = stream.tile([128, SW * 128], dt.bfloat16, tag="m4")
                        m3 = m4[:, :nwf].rearrange("p (w f) -> p w f", w=nw)
                        iv = inv_t[:, w0:w0 + nw].unsqueeze(2).broadcast_to(
                            [128, nw, 128])
                        nc.vector.tensor_tensor(
                            m3, ps_m[:, :nwf].rearrange("p (w f) -> p w f", w=nw),
                            iv, AOp.mult)
                        if debug_dumps and l == 0:
                            nc.sync.dma_start(
                                dbg["dbg_m0"].ap()[w0 * 128:w0 * 128 + nwf, :]
                                    .rearrange("(w p) f -> p w f", w=nw),
                                m3)
                        # mT batched transpose + tm matmuls
                        ps_t4m = pst.tile([128, SW * 128], dt.bfloat16, tag="ps_t")
                        for j in range(nw):
                            nc.tensor.transpose(ps_t4m[:, j * 128:(j + 1) * 128],
                                                m4[:, j * 128:(j + 1) * 128],
                                                ident[:])
                        mt4 = tiny.tile([128, SW * 128], dt.bfloat16, tag="mt")
                        nc.scalar.activation(mt4[:, :nwf], ps_t4m[:, :nwf], AF.Copy)
                        ps_tm = psum.tile([128, SW * CH], dt.float32, tag="ps_sm")
                        for j in range(nw):
                            w = w0 + j
                            nc.tensor.matmul(ps_tm[:, j * CH:(j + 1) * CH],
                                             hT[:, w * 128:(w + 1) * 128],
                                             tmw[l][0][:], start=True, stop=False)
                            nc.tensor.matmul(ps_tm[:, j * CH:(j + 1) * CH],
                                             mt4[:, j * 128:(j + 1) * 128],
                                             tmw[l][1][:],
                                             start=False, stop=True)
                        nwc = nw * CH
                        if ("tmb0", "tmb1")[l] in gb_t:
                            tb = gb_t[("tmb0", "tmb1")[l]][:].unsqueeze(1) \
                                .broadcast_to([128, nw, CH])
                            z3 = ps_tm[:, :nwc].rearrange("p (w c) -> p w c", w=nw)
                            nc.vector.tensor_tensor(z3, z3, tb, AOp.add)
                        # softmax (no max-sub) + cumsum
                        e4 = stream.tile([128, SW * CH], dt.float32, tag="e4")
                        nc.scalar.activation(e4[:, :nwc], ps_tm[:, :nwc], AF.Exp)
                        s4 = small.tile([128, SW], dt.float32, tag="s4")
                        nc.vector.tensor_reduce(
                            s4[:, :nw], e4[:, :nwc].rearrange("p (w c) -> p w c", w=nw),
                            mybir.AxisListType.X, AOp.add)
                        r4s = small.tile([128, SW], dt.float32, tag="r4s")
                        nc.vector.reciprocal(r4s[:, :nw], s4[:, :nw])
                        cs4 = stream.tile([128, SW * CH], dt.float32, tag="cs4")
                        for j in range(nw):
                            nc.vector.tensor_tensor_scan(
                                cs4[:, j * CH:(j + 1) * CH],
                                e4[:, j * CH:(j + 1) * CH],
                                e4[:, j * CH:(j + 1) * CH], 0.0, AOp.add, AOp.bypass)
                        # sig update
                        rb = r4s[:, :nw].unsqueeze(2).broadcast_to([128, nw, CH])
                        cs3 = cs4[:, :nwc].rearrange("p (w c) -> p w c", w=nw)
                        sg_cols = sigc[:, w0 * CH:w0 * CH + nwc]
                        sg3 = sg_cols.rearrange("p (w c) -> p w c", w=nw)
                        if l == 0:
                            nc.vector.tensor_tensor(sg3, cs3, rb, AOp.mult)
                            sig_src = sg_cols
                        else:
                            t4 = stream.tile([128, SW * CH], dt.bfloat16, tag="t4")
                            t3 = t4[:, :nwc].rearrange("p (w c) -> p w c", w=nw)
                            nc.vector.tensor_tensor(t3, cs3, rb, AOp.mult)
                            a4 = stream.tile([128, SW * CH], dt.bfloat16, tag="a4")
                            nc.vector.tensor_tensor(a4[:, :nwc], sg_cols, t4[:, :nwc],
                                                    AOp.mult)
                            nc.vector.tensor_tensor(t4[:, :nwc], t4[:, :nwc],
                                                    a4[:, :nwc], AOp.subtract)
                            nc.vector.tensor_tensor(t4[:, :nwc], t4[:, :nwc],
                                                    sg_cols, AOp.add)
                            sig_src = t4[:, :nwc]
                        # mix u = h*sig + m*(1-sig) = m + sig*(h-m)
                        # sig broadcast over the repeat axis (64 -> 128)
                        sig_b = sig_src.rearrange("p (w c) -> p w c", w=nw) \
                            .unsqueeze(3).broadcast_to([128, nw, CH, 2])
                        hcols = h_own[:, w0 * 128:w0 * 128 + nwf]
                        u4 = stream.tile([128, SW * 128], dt.bfloat16, tag="u4")
                        nc.vector.tensor_tensor(u4[:, :nwf], hcols, m4[:, :nwf],
                                                AOp.subtract)
                        u4d = u4[:, :nwf].rearrange("p (w c r) -> p w c r",
                                                    w=nw, r=2)
                        nc.vector.tensor_tensor(u4d, u4d, sig_b, AOp.mult)
                        nc.vector.tensor_tensor(u4[:, :nwf], u4[:, :nwf], m4[:, :nwf],
                                                AOp.add)
                        # LN stats
                        su = small.tile([128, SW], dt.float32, tag="su")
                        nc.vector.tensor_reduce(
                            su[:, :nw], u4[:, :nwf].rearrange("p (w f) -> p w f", w=nw),
                            mybir.AxisListType.X, AOp.add)
                        sqs = stream.tile([128, SW * 128], dt.bfloat16, tag="e4", name="sqs")
                        nc.scalar.activation(sqs[:, :nwf], u4[:, :nwf], AF.Square)
                        sq = small.tile([128, SW], dt.float32, tag="sq")
                        nc.vector.tensor_reduce(
                            sq[:, :nw], sqs[:, :nwf].rearrange("p (w f) -> p w f", w=nw),
                            mybir.AxisListType.X, AOp.add)
                        mean, rs = ln_smalls(su[:, :nw], sq[:, :nw], nw)
                        gnm = ("lng0", "lng1")[l]
                        bnm = ("lnb0", "lnb1")[l]
                        if l == 0:
                            apply_ln(hcols, u4[:, :nwf], mean, rs, nw,
                                     gnm if gnm in gb_t else None,
                                     bnm if bnm in gb_t else None)
                            nc.sync.dma_start(
                                bounce1.ap()[w0 * 128:w0 * 128 + nwf, :]
                                    .rearrange("(w p) f -> p w f", w=nw),
                                hcols.rearrange("p (w f) -> p w f", w=nw))
                            ps_t4 = pst.tile([128, SW * 128], dt.bfloat16, tag="ps_t")
                            for j in range(nw):
                                w = w0 + j
                                nc.tensor.transpose(
                                    ps_t4[:, j * 128:(j + 1) * 128],
                                    h_own[:, w * 128:(w + 1) * 128], ident[:])
                            nc.scalar.activation(hT[:, w0 * 128:w0 * 128 + nwf],
                                                 ps_t4[:, :nwf], AF.Copy)
                        else:
                            h2 = stream.tile([128, SW * 128], dt.bfloat16, tag="hx", name="h2")
                            apply_ln(h2[:, :nwf], u4[:, :nwf], mean, rs, nw,
                                     gnm if gnm in gb_t else None,
                                     bnm if bnm in gb_t else None)
                            ob = stream.tile([128, SW * OUT_C], dt.float32, tag="ob")
                            ps_o = psum.tile([128, SW * OUT_C], dt.float32, tag="ps_sm")
                            ps_t4 = pst.tile([128, SW * 128], dt.bfloat16, tag="ps_t")
                            for j in range(nw):
                                nc.tensor.transpose(
                                    ps_t4[:, j * 128:(j + 1) * 128],
                                    h2[:, j * 128:(j + 1) * 128], ident[:])
                            h2t4 = tiny.tile([128, SW * 128], dt.bfloat16, tag="h2t")
                            nc.scalar.activation(h2t4[:, :nwf], ps_t4[:, :nwf],
                                                 AF.Copy)
                            for j in range(nw):
                                nc.tensor.matmul(ps_o[:, j * OUT_C:(j + 1) * OUT_C],
                                                 h2t4[:, j * 128:(j + 1) * 128],
                                                 wout[:], start=True, stop=True)
                            nwo = nw * OUT_C
                            if "bout" in gb_t:
                                bb = gb_t["bout"][:].unsqueeze(1).broadcast_to(
                                    [128, nw, OUT_C])
                                o3 = ob[:, :nwo].rearrange("p (w o) -> p w o", w=nw)
                                nc.vector.tensor_tensor(
                                    o3, ps_o[:, :nwo].rearrange("p (w o) -> p w o", w=nw),
                                    bb, AOp.add)
                            else:
                                nc.vector.tensor_copy(ob[:, :nwo], ps_o[:, :nwo])
                            nc.sync.dma_start(
                                out_d.ap()[w0 * 128:w0 * 128 + nwf, :]
                                    .rearrange("(w p) o -> p w o", w=nw),
                                ob[:, :nwo].rearrange("p (w o) -> p w o", w=nw))
                    if l == 0:
                        _ag(bounce1, hfull1)
                    if l == 0 and debug_dumps:
                        nc.sync.dma_start(dbg["dbg_sig0"].ap(), sigc[:])
                        nc.sync.dma_start(
                            dbg["dbg_h1"].ap().rearrange("(w p) f -> p w f", w=WPC),
                            h_own[:].rearrange("p (w f) -> p w f", w=WPC))


            for _rep in range(repeat):
                _once()

    nc.compile()
    return nc


_CACHE = {}


def _sched_key(prep):
    return (tuple(prep["tile_win"]),
            tuple((c, t0, tuple(wr)) for sw in prep["sched"]
                  for c, t0, wr in sw))


def _get_compiled(cfg, prep, flags):
    key = (_sched_key(prep), tuple(sorted(flags.items())))
    if key not in _CACHE:
        _CACHE[key] = build_nc(cfg, prep, flags)
    return _CACHE[key]


class PjrtRunner:
    """Persistent jitted shard_map executor for one compiled nc (8 cores)."""

    def __init__(self, nc, donate=True):
        import jax
        from jax.experimental.shard_map import shard_map
        from jax.sharding import Mesh, PartitionSpec
        from concourse import bass2jax

        bass2jax.install_neuronx_cc_hook()
        self.nc = nc
        in_names, out_names, out_avals, zero_outs = [], [], [], []
        partition_name = (nc.partition_id_tensor.name
                          if nc.partition_id_tensor else None)
        for alloc in nc.m.functions[0].allocations:
            if not isinstance(alloc, mybir.MemoryLocationSet):
                continue
            name = alloc.memorylocations[0].name
            if alloc.kind == "ExternalInput":
                if name != partition_name:
                    in_names.append(name)
            elif alloc.kind == "ExternalOutput":
                import jax.core as jcore
                out_names.append(name)
                aval = jax.core.ShapedArray(
                    tuple(alloc.tensor_shape), mybir.dt.np(alloc.dtype))
                out_avals.append(aval)
                zero_outs.append(np.zeros(alloc.tensor_shape,
                                          mybir.dt.np(alloc.dtype)))
        self.n_params = len(in_names)
        self.out_names = list(out_names)
        self.zero_outs = zero_outs
        all_in = in_names + out_names
        if partition_name is not None:
            all_in.append(partition_name)
        self.in_names_data = in_names

        def _body(*args):
            operands = list(args)
            if partition_name is not None:
                operands.append(bass2jax.partition_id_tensor())
            outs = bass2jax._bass_exec_p.bind(
                *operands,
                out_avals=tuple(out_avals),
                in_names=tuple(all_in),
                out_names=tuple(out_names),
                lowering_input_output_aliases=(),
                sim_require_finite=True,
                sim_require_nnan=True,
                nc=nc,
            )
            return tuple(outs)

        devices = jax.devices()[:NCORES]
        self.mesh = Mesh(np.asarray(devices), ("core",))
        n_out = len(out_names)
        donate_nums = (tuple(range(self.n_params, self.n_params + n_out))
                       if donate else ())
        in_specs = (PartitionSpec("core"),) * (self.n_params + n_out)
        out_specs = (PartitionSpec("core"),) * n_out
        self.fn = jax.jit(
            shard_map(_body, mesh=self.mesh, in_specs=in_specs,
                      out_specs=out_specs, check_rep=False),
            donate_argnums=donate_nums, keep_unused=True)

    def concat_inputs(self, in_maps):
        return [
            np.concatenate([np.asarray(in_maps[c][nm]) for c in range(NCORES)],
                           axis=0)
            for nm in self.in_names_data
        ]

    def zeros(self):
        return [np.zeros((NCORES * z.shape[0], *z.shape[1:]), z.dtype)
                for z in self.zero_outs]

    def __call__(self, concat_in, zeros):
        import jax
        outs = self.fn(*concat_in, *zeros)
        return {nm: np.asarray(outs[i]) for i, nm in enumerate(self.out_names)}


_RUNNERS = {}


def get_runner(cfg, prep, flags):
    key = (_sched_key(prep), tuple(sorted(flags.items())))
    if key not in _RUNNERS:
        _RUNNERS[key] = PjrtRunner(_get_compiled(cfg, prep, flags))
    return _RUNNERS[key]


def run(inputs, cfg):
    x = np.asarray(inputs["x"], np.float32)
    prep = _host_prep(x, np.asarray(inputs["edge_index"]), cfg)
    SH = cfg["SH"]

    flags = make_flags(inputs)
    runner = get_runner(cfg, prep, flags)
    in_maps = make_in_maps(inputs, prep, flags)
    out = runner(runner.concat_inputs(in_maps), runner.zeros())["out"]
    SHP = prep["SHP"]
    out = out.reshape(NCORES, SHP, OUT_C)[:, :SH, :]
    return np.ascontiguousarray(out.reshape(NCORES * SH, OUT_C), dtype=np.float32)


def make_flags(inputs):
    return {
        "bin_triv": _affine_trivial(1.0, inputs["b_in"]),
        "gin_triv": _affine_trivial(inputs["g_in"], 0.0),
        "bein_triv": _affine_trivial(1.0, inputs["be_in"]),
        "lng0_triv": _affine_trivial(inputs["ln_g0"], 0.0),
        "lnb0_triv": _affine_trivial(1.0, inputs["ln_b0"]),
        "lng1_triv": _affine_trivial(inputs["ln_g1"], 0.0),
        "lnb1_triv": _affine_trivial(1.0, inputs["ln_b1"]),
        "tmb0_triv": _affine_trivial(1.0, inputs["tm_b0"]),
        "tmb1_triv": _affine_trivial(1.0, inputs["tm_b1"]),
        "bout_triv": _affine_trivial(1.0, inputs["b_out"]),
    }


def make_in_maps(inputs, prep, flags):
    def bc(v, width):
        return np.tile(np.asarray(v, np.float32).reshape(1, width), (128, 1))

    in_maps = []
    for k in range(NCORES):
        import ml_dtypes
        bf16 = ml_dtypes.bfloat16
        m = {
            "xT": prep["xT"][k].astype(bf16),
            "w_in": np.asarray(inputs["W_in"], np.float32).astype(bf16),
            "idx16": prep["idx16"][k],
            "rel": prep["rel"][k].astype(bf16),
            "inv": prep["inv"][k],
            "tmw0": np.asarray(inputs["tm_W0"], np.float32).astype(bf16),
            "tmw1": np.asarray(inputs["tm_W1"], np.float32).astype(bf16),
            "wout": np.asarray(inputs["W_out"], np.float32).astype(bf16),
            "iota": np.tile(np.arange(128, dtype=np.float32)[None, :],
                            (128, 1)).astype(bf16),
            "ident": np.eye(128, dtype=np.float32).astype(bf16),
        }
        if not flags["bin_triv"]:
            m["bin"] = bc(inputs["b_in"], F)
        if not flags["gin_triv"]:
            m["gin"] = bc(inputs["g_in"], F)
        if not flags["bein_triv"]:
            m["bein"] = bc(inputs["be_in"], F)
        for nm, src in (("lng0", "ln_g0"), ("lnb0", "ln_b0"),
                        ("lng1", "ln_g1"), ("lnb1", "ln_b1")):
            if not flags[nm + "_triv"]:
                m[nm] = bc(inputs[src], F)
        if not flags["tmb0_triv"]:
            m["tmb0"] = bc(inputs["tm_b0"], CH)
        if not flags["tmb1_triv"]:
            m["tmb1"] = bc(inputs["tm_b1"], CH)
        if not flags["bout_triv"]:
            m["bout"] = bc(inputs["b_out"], OUT_C)
        in_maps.append(m)
    return in_maps


def kernel(**inputs):
    return run(inputs, FULL_CFG)

